# revision 1
# baseline (speedup 1.0000x reference)
"""Trainium2 Bass kernel for nn_CATransformer1 (XCiT-style channel-attention block).

Sharding: data-parallel over batch. 16 images / 8 cores = 2 images per core.
Weights are replicated; no collectives. Each core computes its 2 images fully.

Math (per image, x [C=384, N=4096]):
  LN1 is folded into the QKV matmul:
      qkT[n,j] = rstd_n * ( sum_c x[c,n] Wqk[j,c]  - m_n * u[j] )
  with the "-m_n*u[j]" rank-1 term realized as an extra K=1 matmul row
  (lhsT=mneg_row, rhs=u_row) accumulated into the same PSUM, and the
  per-pixel rstd_n applied at PSUM eviction (pixels are PSUM partitions).
  q,k are produced in pixel-partition layout [N, 48] per head, which is
  exactly what the channel-attention S = qn@kn^T (contraction over N)
  needs as lhsT/rhs.  L2 norms of q,k rows are computed with ones-vector
  matmuls from squared tiles.
  The attention output + projection is algebraically collapsed into a
  per-image 384x384 matrix  G = Wproj @ concat_h(attn_h @ Wv_h), so the whole
  attention branch output is:
      attn_branch[o,n] = rstd_n * ( (G @ x)[o,n] - m_n * uG[o] )
  again via the rank-1 augmentation + a row-broadcast rstd multiply.
  FFN: LN2 computed explicitly per 256-col chunk (stats via ones-matmuls),
  yn materialized per-chunk, ffn1 with fused GELU eviction on the scalar
  engine, ffn2 with fused residual-add eviction on the vector engine.
  All large matmuls use float32r (full-rate fp32, ~1.4e-4 rel err); see
  _split_waits/_patch_tile_drain for required walrus workarounds.
"""

import numpy as np

B, C, NH, CH, N, HID = 16, 384, 8, 48, 4096, 1536
NCORES = 8
BPC = B // NCORES  # images per core
P = 128
KS = C // P  # 3 k-subtiles for C
KH = HID // P  # 12 k-subtiles for HID
LOGIT_MAX = float(np.log(1.0 / 0.01))
EPS_LN = 1e-5
EPS_NORM = 1e-12

_CACHE = {}


def _patch_tile_drain():
    """Walrus in this env rejects >1 sync-wait on the kernel-tail Drain
    (CTRL_NO_STRUCT setupSyncWait).  Split the waits across a chain of
    drain instructions, one wait each.  Idempotent, in-process only."""
    import concourse.tile as tile
    from concourse import mybir
    from concourse.vector_clock import ScopedClock

    if getattr(tile.TileContext._drain_and_barrier, "_split_patch", False):
        return

    def _split_drain(self, tick_clock, wait_clock):
        drain_inst = self.nc.sync.drain()
        wait_clock.add_sem_waits(
            drain_inst.ins, ScopedClock({None: tick_clock.global_clock}))
        si = drain_inst.ins.sync_info
        if si is not None and si.on_wait and len(si.on_wait) > 1:
            waits = list(si.on_wait)
            si.on_wait = waits[:1]
            for w in waits[1:]:
                d2 = self.nc.sync.drain()
                d2.ins.sync_info = mybir.SyncInfo(on_wait=[w], on_update=[])
        self.nc.all_engine_barrier()
        popped = self.nc._tile_sem_poison_stack.pop()
        assert popped is self._sem_poison
        self.nc.clear_and_free_semaphores(list(self.sems.allocated().values()))
        self.nc.all_engine_barrier()

    _split_drain._split_patch = True
    tile.TileContext._drain_and_barrier = _split_drain


def _split_waits(nc, max_waits=1):
    """This walrus build rejects instructions carrying more than one sync
    wait ('Too many sync wait commands' / 'ISA wrong length').  Move extra
    waits onto same-engine NoOps inserted immediately before."""
    from concourse import mybir

    n = 0
    for fn in nc.m.functions:
        for blk in fn.blocks:
            out = []
            for inst in blk.instructions:
                si = inst.sync_info
                if si is not None and si.on_wait and len(si.on_wait) > max_waits:
                    waits = list(si.on_wait)
                    for w in waits[:-max_waits]:
                        n += 1
                        nop = mybir.InstNoOp(
                            name=f"I-wsplit-{n}", ins=[], outs=[])
                        nop.engine = inst.engine
                        nop.sync_info = mybir.SyncInfo(
                            on_wait=[w], on_update=[])
                        out.append(nop)
                    si.on_wait = waits[-max_waits:]
                out.append(inst)
            blk.instructions = out
    return nc


def _build_nc():
    import concourse.bass as bass
    import concourse.tile as tile
    from concourse import mybir

    dt = mybir.dt
    AF = mybir.ActivationFunctionType
    ALU = mybir.AluOpType
    AX = mybir.AxisListType
    from concourse.masks import make_identity

    f32 = dt.float32
    f32r = dt.float32r  # full-rate fp32 matmul dtype (~1.4e-4 rel err)

    _patch_tile_drain()
    nc = bass.Bass()

    xs = nc.declare_dram_parameter("xs", [BPC, C, N], f32, isOutput=False)
    wqk_t = nc.declare_dram_parameter("wqk_t", [C, 2 * C], f32, isOutput=False)
    u_qk = nc.declare_dram_parameter("u_qk", [1, 2 * C], f32, isOutput=False)
    wv = nc.declare_dram_parameter("wv", [CH, NH, C], f32, isOutput=False)
    wpj48 = nc.declare_dram_parameter("wpj48", [CH, NH, C], f32, isOutput=False)
    w1_t = nc.declare_dram_parameter("w1_t", [C, HID], f32, isOutput=False)
    w2_t = nc.declare_dram_parameter("w2_t", [HID, C], f32, isOutput=False)
    scale_row = nc.declare_dram_parameter("scale_row", [1, NH], f32, isOutput=False)
    out_d = nc.declare_dram_parameter("out", [BPC, C, N], f32, isOutput=True)

    FC = 256   # stats+qk pixel chunk
    NFC = N // FC
    FG = 256   # G-pass / ffn pixel chunk
    NFG = N // FG
    NT = N // P  # 128-pixel chunks

    with tile.TileContext(nc) as tc:
        with (
            tc.tile_pool(name="consts", bufs=1) as consts,
            tc.tile_pool(name="xc", bufs=2) as xcp,
            tc.tile_pool(name="xg", bufs=2) as xgp,
            tc.tile_pool(name="qk", bufs=2) as qkpool,
            tc.tile_pool(name="attn", bufs=1) as apool,
            tc.tile_pool(name="gt", bufs=1) as gtp,
            tc.tile_pool(name="workA", bufs=2) as work,
            tc.tile_pool(name="hb", bufs=1) as hbp,
            tc.tile_pool(name="small", bufs=2) as small,
            tc.tile_pool(name="ps", bufs=5, space="PSUM") as ps,
            tc.tile_pool(name="psacc", bufs=1, space="PSUM") as psacc,
            tc.tile_pool(name="dram", bufs=2, space="DRAM") as dramp,
        ):
            def bcast_read(dst, dram_row, parts=P):
                """DMA a DRAM row [F] into dst [parts,F] replicated across
                partitions (stride-0 partition dim)."""
                src = bass.AP(
                    tensor=dram_row.tensor, offset=dram_row.offset,
                    ap=[[0, parts]] + [list(d) for d in dram_row.ap[-1:]])
                nc.gpsimd.dma_start(dst, src)

            # ------------- constants (cast to f32r via gpsimd DMA) -------
            wqk_sb = consts.tile([P, KS, 2 * C], f32r, tag="wqk")
            nc.gpsimd.dma_start(wqk_sb[:], wqk_t.rearrange("(s p) f -> p s f", p=P))
            wv_sb = consts.tile([CH, NH, C], f32r, tag="wv")
            nc.gpsimd.dma_start(wv_sb[:], wv[:])
            wpj_sb = consts.tile([CH, NH, C], f32r, tag="wpj")
            nc.gpsimd.dma_start(wpj_sb[:], wpj48[:])
            w1_sb = consts.tile([P, KS, HID], f32r, tag="w1")
            nc.gpsimd.dma_start(w1_sb[:], w1_t.rearrange("(s p) f -> p s f", p=P))
            w2_sb = consts.tile([P, KH, C], f32r, tag="w2")
            nc.gpsimd.dma_start(w2_sb[:], w2_t.rearrange("(s p) f -> p s f", p=P))
            uqk_sb = consts.tile([1, 2 * C], f32r, tag="uqk")
            nc.gpsimd.dma_start(uqk_sb[:], u_qk[:])
            ones_c = consts.tile([P, KS, 1], f32, tag="ones")
            nc.vector.memset(ones_c[:], 1.0)
            ones_r = consts.tile([P, KS, 1], f32r, tag="onesr")
            nc.vector.tensor_copy(ones_r[:], ones_c[:])
            ones2_c = consts.tile([P, 2], f32, tag="ones2")
            nc.vector.memset(ones2_c[:], 1.0)
            ones2_r = consts.tile([P, 2], f32r, tag="ones2r")
            nc.vector.tensor_copy(ones2_r[:], ones2_c[:])
            onesrow_c = consts.tile([1, P], f32, tag="onesrow")
            nc.vector.memset(onesrow_c[:], 1.0)
            onesrow_r = consts.tile([1, P], f32r, tag="onesrowr")
            nc.vector.tensor_copy(onesrow_r[:], onesrow_c[:])
            ident = consts.tile([CH, CH], f32, tag="ident")
            make_identity(nc, ident[:])
            schb = consts.tile([CH, NH], f32, tag="schb")
            bcast_read(schb[:], scale_row[0, :], parts=CH)

            xs_r = xs.rearrange("b (s p) n -> b p s n", p=P)
            out_r = out_d.rearrange("b (s p) n -> b p s n", p=P)

            for img in range(BPC):
                mneg_dram = dramp.tile([1, N], f32r, tag="mnegdram")
                rstd_dram = dramp.tile([1, N], f32, tag="rstddram")

                # ---- pass A: LN1 stats + qkT + S/norm accumulation ----
                ps_s = psacc.tile([CH, NH * CH], f32, tag="psS")
                ps_nq = psacc.tile([CH, 2 * NH], f32, tag="psnq")
                ps_nk = psacc.tile([1, C], f32, tag="psnk")
                for f in range(NFC):
                    sl = slice(f * FC, (f + 1) * FC)
                    xc = xcp.tile([P, KS, FC], f32, tag="xc")
                    nc.sync.dma_start(xc[:], xs_r[img][:, :, sl])
                    xcr = xcp.tile([P, KS, FC], f32r, tag="xcr")
                    nc.gpsimd.dma_start(xcr[:], xs_r[img][:, :, sl])
                    xsqr = work.tile([P, KS, FC], f32r, tag="xsq")
                    nc.vector.tensor_mul(xsqr[:], xc[:], xc[:])
                    prow = ps.tile([1, 2 * FC], f32, tag="pb")
                    for s in range(KS):
                        nc.tensor.matmul(
                            prow[0:1, 0:FC], ones_r[:, s, :], xcr[:, s, :],
                            start=(s == 0), stop=(s == KS - 1))
                    for s in range(KS):
                        nc.tensor.matmul(
                            prow[0:1, FC:], ones_r[:, s, :], xsqr[:, s, :],
                            start=(s == 0), stop=(s == KS - 1))
                    mneg_f = small.tile([1, FC], f32, tag="mnegf")
                    nc.vector.tensor_scalar(
                        mneg_f[:], prow[0:1, 0:FC], -1.0 / C, None, op0=ALU.mult)
                    mneg_t = small.tile([1, FC], f32r, tag="mnegt")
                    nc.vector.tensor_copy(mneg_t[:], mneg_f[:])
                    nc.sync.dma_start(mneg_dram[0:1, sl], mneg_t[:])
                    # var+eps = E[x^2]+eps - (sum x)^2/C^2  (all reads f32)
                    vrow = small.tile([1, FC], f32, tag="vrow")
                    nc.vector.tensor_scalar(
                        vrow[:], prow[0:1, FC:], 1.0 / C, EPS_LN,
                        op0=ALU.mult, op1=ALU.add)
                    msq = small.tile([1, FC], f32, tag="msq")
                    nc.vector.tensor_mul(msq[:], mneg_f[:], mneg_f[:])
                    nc.vector.tensor_sub(vrow[:], vrow[:], msq[:])
                    rrow = small.tile([1, FC], f32, tag="rrow")
                    nc.scalar.activation(rrow[:], vrow[:], AF.Sqrt)
                    nc.vector.reciprocal(rrow[:], rrow[:])
                    nc.sync.dma_start(rstd_dram[0:1, sl], rrow[:])
                    # independent column-form stats for the 2 pixel chunks
                    # (avoids serializing qk evictions behind the row chain)
                    pcol = ps.tile([P, 2, 2, 2], f32, tag="pb")
                    for t in range(2):
                        tsl = slice(t * P, (t + 1) * P)
                        for s in range(KS):
                            nc.tensor.matmul(
                                pcol[:, 0, t, :], xcr[:, s, tsl],
                                ones2_r[:, :], start=(s == 0), stop=(s == KS - 1))
                        for s in range(KS):
                            nc.tensor.matmul(
                                pcol[:, 1, t, :], xsqr[:, s, tsl],
                                ones2_r[:, :], start=(s == 0), stop=(s == KS - 1))
                    mcol = small.tile([P, 2, 2], f32, tag="mcol")
                    nc.vector.tensor_scalar(
                        mcol[:], pcol[:, :, :, 0], 1.0 / C, None, op0=ALU.mult)
                    vcol = small.tile([P, 2], f32, tag="vcol")
                    nc.vector.tensor_mul(vcol[:], mcol[:, 0, :], mcol[:, 0, :])
                    nc.vector.tensor_sub(vcol[:], mcol[:, 1, :], vcol[:])
                    nc.vector.tensor_scalar(
                        vcol[:], vcol[:], EPS_LN, None, op0=ALU.add)
                    rcol = small.tile([P, 2], f32, tag="rcol")
                    nc.scalar.activation(rcol[:], vcol[:], AF.Sqrt)
                    nc.vector.reciprocal(rcol[:], rcol[:])

                    for t in range(2):
                        tt = f * 2 + t  # global 128-pixel chunk
                        tsl = slice(t * P, (t + 1) * P)
                        pa = ps.tile([P, 512], f32, tag="pb")
                        pb = ps.tile([P, 256], f32, tag="pb")
                        for s in range(KS):
                            nc.tensor.matmul(
                                pa[:], xcr[:, s, tsl], wqk_sb[:, s, 0:512],
                                start=(s == 0), stop=False)
                        nc.tensor.matmul(
                            pa[:], mneg_t[0:1, tsl], uqk_sb[:, 0:512],
                            start=False, stop=True)
                        for s in range(KS):
                            nc.tensor.matmul(
                                pb[:], xcr[:, s, tsl], wqk_sb[:, s, 512:768],
                                start=(s == 0), stop=False)
                        nc.tensor.matmul(
                            pb[:], mneg_t[0:1, tsl], uqk_sb[:, 512:768],
                            start=False, stop=True)
                        qk = qkpool.tile([P, 2 * C], f32, tag="qk")
                        qksq = qkpool.tile([P, 2 * C], f32r, tag="qksq")
                        rc = rcol[:, t : t + 1]
                        nc.vector.tensor_scalar_mul(qk[:, 0:512], pa[:], rc)
                        nc.vector.tensor_scalar_mul(qk[:, 512:768], pb[:], rc)
                        nc.vector.tensor_mul(qksq[:], qk[:], qk[:])
                        st, sp = (tt == 0), (tt == NT - 1)
                        for h in range(NH):
                            o = h * 2 * CH
                            nc.tensor.matmul(
                                ps_s[:, h * CH : (h + 1) * CH],
                                qk[:, o : o + CH], qk[:, o + CH : o + 2 * CH],
                                start=st, stop=sp)
                            nc.tensor.matmul(
                                ps_nq[:, 2 * h : 2 * h + 2],
                                qksq[:, o : o + CH], ones2_r[:, :],
                                start=st, stop=sp)
                        ksq = qksq.rearrange(
                            "p (h two c) -> p h two c", two=2, c=CH)
                        nc.tensor.matmul(
                            ps_nk[:], ones_r[:, 0, :], ksq[:, :, 1, :],
                            start=st, stop=sp)

                # ---------------- attn softmax + G build ----------------
                rq = apool.tile([CH, NH], f32, tag="rq")
                nc.scalar.activation(
                    rq[:], ps_nq.rearrange("p (h two) -> p h two", two=2)[:, :, 0],
                    AF.Sqrt)
                nc.vector.tensor_scalar_max(rq[:], rq[:], EPS_NORM)
                nc.vector.reciprocal(rq[:], rq[:])
                nc.vector.tensor_mul(rq[:], rq[:], schb[:])  # * exp(logit_scale)
                rk = apool.tile([1, C], f32, tag="rk")
                nc.scalar.activation(rk[:], ps_nk[:], AF.Sqrt)
                nc.vector.tensor_scalar_max(rk[:], rk[:], EPS_NORM)
                nc.vector.reciprocal(rk[:], rk[:])
                rk_r = apool.tile([1, C], f32r, tag="rkr")
                nc.vector.tensor_copy(rk_r[:], rk[:])
                rkb_ps = ps.tile([CH, C], f32, tag="pb")
                nc.tensor.matmul(
                    rkb_ps[:], onesrow_r[0:1, :CH], rk_r[0:1, :],
                    start=True, stop=True)
                sS = apool.tile([CH, C], f32, tag="sS")
                for h in range(NH):
                    hs = slice(h * CH, (h + 1) * CH)
                    nc.vector.tensor_scalar_mul(
                        sS[:, hs], ps_s[:CH, hs], rq[:, h : h + 1])
                nc.vector.tensor_mul(sS[:], sS[:], rkb_ps[:])
                mx = apool.tile([CH, NH], f32, tag="mx")
                esum = apool.tile([CH, NH], f32, tag="esum")
                for h in range(NH):
                    hs = slice(h * CH, (h + 1) * CH)
                    nc.vector.reduce_max(mx[:, h : h + 1], sS[:, hs], axis=AX.X)
                    nc.vector.tensor_scalar(
                        sS[:, hs], sS[:, hs], mx[:, h : h + 1], None,
                        op0=ALU.subtract)
                    nc.scalar.activation(
                        sS[:, hs], sS[:, hs], AF.Exp,
                        accum_out=esum[:, h : h + 1])
                nc.vector.reciprocal(esum[:], esum[:])
                for h in range(NH):
                    hs = slice(h * CH, (h + 1) * CH)
                    nc.vector.tensor_scalar_mul(
                        sS[:, hs], sS[:, hs], esum[:, h : h + 1])
                atT = apool.tile([CH, C], f32r, tag="atT")
                for h in range(NH):
                    hs = slice(h * CH, (h + 1) * CH)
                    ptr = ps.tile([CH, CH], f32, tag="pb")
                    nc.tensor.transpose(ptr[:], sS[:, hs], ident[:])
                    nc.vector.tensor_copy(atT[:, hs], ptr[:])
                awv_sb = apool.tile([CH, NH, C], f32r, tag="awv")
                for h in range(NH):
                    paw = ps.tile([CH, C], f32, tag="pb")
                    nc.tensor.matmul(
                        paw[:], atT[:, h * CH : (h + 1) * CH],
                        wv_sb[:, h, :], start=True, stop=True)
                    nc.vector.tensor_copy(awv_sb[:, h, :], paw[:])
                # G^T[C', o] = sum_{h,d} awv[d,h,C'] * wproj[o, 48h+d]
                gt_sb = gtp.tile([P, KS, C], f32r, tag="gt")
                for j in range(KS):
                    pgt = ps.tile([P, C], f32, tag="pb")
                    for h in range(NH):
                        nc.tensor.matmul(
                            pgt[:], awv_sb[:, h, j * P : (j + 1) * P],
                            wpj_sb[:, h, :], start=(h == 0), stop=(h == NH - 1))
                    nc.vector.tensor_copy(gt_sb[:, j, :], pgt[:])
                ug = gtp.tile([1, C], f32r, tag="ug")
                pug = ps.tile([1, C], f32, tag="pb")
                for s in range(KS):
                    nc.tensor.matmul(
                        pug[:], ones_r[:, s, :], gt_sb[:, s, :],
                        start=(s == 0), stop=(s == KS - 1))
                nc.vector.tensor_copy(ug[:], pug[:])

                # ---- pass B: attn branch + residual + LN2 + FFN ----
                for f in range(NFG):
                    sl = slice(f * FG, (f + 1) * FG)
                    xg = xgp.tile([P, KS, FG], f32, tag="xg")
                    nc.sync.dma_start(xg[:], xs_r[img][:, :, sl])
                    xgr = xgp.tile([P, KS, FG], f32r, tag="xgr")
                    nc.gpsimd.dma_start(xgr[:], xs_r[img][:, :, sl])
                    mneg_g = small.tile([1, FG], f32r, tag="mnegg")
                    nc.sync.dma_start(mneg_g[:], mneg_dram[0:1, sl])
                    rb = work.tile([P, FG], f32, tag="rb")
                    bcast_read(rb[:], rstd_dram[0, sl])
                    y = work.tile([P, KS, FG], f32, tag="y")
                    for j in range(KS):
                        pg = ps.tile([P, FG], f32, tag="pb")
                        for s in range(KS):
                            nc.tensor.matmul(
                                pg[:], gt_sb[:, s, j * P : (j + 1) * P],
                                xgr[:, s, :], start=(s == 0), stop=False)
                        nc.tensor.matmul(
                            pg[:], ug[:, j * P : (j + 1) * P],
                            mneg_g[:], start=False, stop=True)
                        ab = work.tile([P, FG], f32, tag="ab")
                        nc.vector.tensor_mul(ab[:], pg[:], rb[:])
                        nc.vector.tensor_add(y[:, j, :], xg[:, j, :], ab[:])
                    # LN2 stats for this chunk
                    yr = work.tile([P, KS, FG], f32r, tag="yr")
                    nc.vector.tensor_copy(yr[:], y[:])
                    ysqr = work.tile([P, KS, FG], f32r, tag="xsq")
                    nc.scalar.activation(ysqr[:], y[:], AF.Square)
                    p2 = ps.tile([1, 2 * FG], f32, tag="pb")
                    for s in range(KS):
                        nc.tensor.matmul(
                            p2[0:1, 0:FG], ones_r[:, s, :], yr[:, s, :],
                            start=(s == 0), stop=(s == KS - 1))
                    for s in range(KS):
                        nc.tensor.matmul(
                            p2[0:1, FG:], ones_r[:, s, :], ysqr[:, s, :],
                            start=(s == 0), stop=(s == KS - 1))
                    m2_f = small.tile([1, FG], f32, tag="m2")
                    nc.vector.tensor_scalar(
                        m2_f[:], p2[0:1, 0:FG], -1.0 / C, None, op0=ALU.mult)
                    m2r2 = small.tile([1, 2 * FG], f32r, tag="m2r2")
                    nc.vector.tensor_copy(m2r2[0:1, 0:FG], m2_f[:])
                    v2 = small.tile([1, FG], f32, tag="vrow")
                    nc.vector.tensor_scalar(
                        v2[:], p2[0:1, FG:], 1.0 / C, EPS_LN,
                        op0=ALU.mult, op1=ALU.add)
                    msq2 = small.tile([1, FG], f32, tag="msq")
                    nc.vector.tensor_mul(msq2[:], m2_f[:], m2_f[:])
                    nc.vector.tensor_sub(v2[:], v2[:], msq2[:])
                    r2 = small.tile([1, FG], f32, tag="r2")
                    nc.scalar.activation(r2[:], v2[:], AF.Sqrt)
                    nc.vector.reciprocal(r2[:], r2[:])
                    nc.vector.tensor_copy(m2r2[0:1, FG:], r2[:])
                    bc_ps = ps.tile([P, 2 * FG], f32, tag="pb")
                    nc.tensor.matmul(
                        bc_ps[:], onesrow_r[0:1, :], m2r2[0:1, :],
                        start=True, stop=True)
                    t3 = work.tile([P, KS, FG], f32, tag="t3")
                    nc.vector.tensor_add(
                        t3[:], y[:],
                        bc_ps[:, None, 0:FG].to_broadcast((P, KS, FG)))
                    yn = work.tile([P, KS, FG], f32r, tag="yn")
                    nc.vector.tensor_mul(
                        yn[:], t3[:],
                        bc_ps[:, None, FG:].to_broadcast((P, KS, FG)))
                    # ffn1 + gelu
                    h_sb = hbp.tile([P, KH, FG], f32r, tag="h")
                    for mh in range(KH):
                        ph = ps.tile([P, FG], f32, tag="pb")
                        for s in range(KS):
                            nc.tensor.matmul(
                                ph[:], w1_sb[:, s, mh * P : (mh + 1) * P],
                                yn[:, s, :], start=(s == 0), stop=(s == KS - 1))
                        nc.scalar.activation(h_sb[:, mh, :], ph[:], AF.Gelu)
                    # ffn2 + residual (in place into y), then store
                    for mo in range(KS):
                        po = ps.tile([P, FG], f32, tag="pb")
                        for s in range(KH):
                            nc.tensor.matmul(
                                po[:], w2_sb[:, s, mo * P : (mo + 1) * P],
                                h_sb[:, s, :],
                                start=(s == 0), stop=(s == KH - 1))
                        nc.vector.tensor_add(y[:, mo, :], po[:], y[:, mo, :])
                    nc.sync.dma_start(out_r[img][:, :, sl], y[:])
    return _split_waits(nc)


def _prep_weights(inputs):
    w_qkv = np.asarray(inputs["w_qkv"], np.float32)
    g1 = np.asarray(inputs["g1"], np.float32)
    g2 = np.asarray(inputs["g2"], np.float32)
    for name in ("beta1", "beta2", "b_qkv", "b_proj", "b_ffn1", "b_ffn2"):
        assert not np.any(np.asarray(inputs[name])), f"{name} nonzero unsupported"
    wg = w_qkv * g1[None, :]  # fold LN gamma into qkv weights
    wg3 = wg.reshape(NH, 3 * CH, C)
    wq = wg3[:, 0:CH, :]  # [NH, 48, C]
    wk = wg3[:, CH : 2 * CH, :]
    wv_ = wg3[:, 2 * CH : 3 * CH, :]
    # qk columns interleaved per head: j = h*96 + (0..47 q | 48..95 k)
    wqk = np.concatenate([wq, wk], axis=1).reshape(2 * C, C)  # [768, 384]
    wqk_t = np.ascontiguousarray(wqk.T)  # [384, 768]
    u_qk = wqk.sum(axis=1)[None, :]  # [1, 768]
    wv_t = np.ascontiguousarray(wv_.transpose(1, 0, 2))  # [48, NH, 384]
    # wpj48[d, h, o] = w_proj[o, 48h+d]
    wpj48 = np.ascontiguousarray(
        np.asarray(inputs["w_proj"], np.float32).T.reshape(NH, CH, C)
        .transpose(1, 0, 2))
    w1g = np.asarray(inputs["w_ffn1"], np.float32) * g2[None, :]
    w1_t = np.ascontiguousarray(w1g.T)  # [384, 1536]
    w2_t = np.ascontiguousarray(np.asarray(inputs["w_ffn2"], np.float32).T)
    ls = np.asarray(inputs["logit_scale"], np.float32).reshape(NH)
    scale_row = np.exp(np.minimum(ls, LOGIT_MAX))[None, :]
    return dict(
        wqk_t=wqk_t, u_qk=np.ascontiguousarray(u_qk), wv=wv_t,
        wpj48=wpj48, w1_t=w1_t, w2_t=w2_t,
        scale_row=np.ascontiguousarray(scale_row))


def kernel(**inputs):
    from concourse.bass_utils import run_bass_kernel_spmd

    if "nc" not in _CACHE:
        _CACHE["nc"] = _build_nc()
    nc = _CACHE["nc"]

    x = np.asarray(inputs["x"], np.float32).reshape(B, C, N)
    wmap = _prep_weights(inputs)
    in_maps = []
    for c in range(NCORES):
        m = dict(wmap)
        m["xs"] = np.ascontiguousarray(x[c * BPC : (c + 1) * BPC])
        in_maps.append(m)
    res = run_bass_kernel_spmd(nc, in_maps, list(range(NCORES)))
    out = np.concatenate([r["out"] for r in res.results], axis=0)
    return out.reshape(B, C, 64, 64).astype(np.float32)



# revision 10
# speedup vs baseline: 1.0693x; 1.0693x over previous
"""Trainium2 Bass kernel for nn_CATransformer1 (XCiT-style channel-attention block).

Sharding: data-parallel over batch. 16 images / 8 cores = 2 images per core.
Weights replicated; no collectives.

V2 design (bf16 everywhere):
  - x is DMAed once per image (bf16) and stays SBUF-resident across both
    passes; output written back as bf16 and widened on host.
  - All matmuls run in bf16 (full rate at any free-dim size on TRN2).
  - LN1/LN2 stats are computed in column form (per-pixel partitions) with
    F=1 ones-matmuls (nearly free on the PE), then transposed to row form
    for the rank-1 mean terms and row-broadcasts.
  - LN1 mean is folded into the QKV matmul as a rank-1 K=1 accumulation
    (lhsT=mneg row, rhs=u row); rstd applied at PSUM eviction via
    per-partition tensor_scalar (pixels are partitions).
  - Attention output + projection collapsed into per-image G = Wproj @
    concat_h(attn_h @ Wv_h); attn branch = rstd * (G@x - m*uG) via the same
    rank-1 trick; LN2 materializes yn (bf16) for the FFN.
  - Eviction work split across DVE and Activation engines; emission is
    software-pipelined (S-accum deferred one chunk; image-1 attention block
    interleaved into image-0 phase B) so the PE stays fed.
"""

import numpy as np

B, C, NH, CH, N, HID = 16, 384, 8, 48, 4096, 1536
NCORES = 8
BPC = B // NCORES  # images per core
P = 128
KS = C // P   # 3 k-subtiles for C
KH = HID // P  # 12 k-subtiles for HID
NT = N // P   # 32 pixel chunks (phase A)
FG = 256      # phase B pixel chunk
NFG = N // FG
LOGIT_MAX = float(np.log(1.0 / 0.01))
EPS_LN = 1e-5
EPS_NORM = 1e-12

_CACHE = {}


def _patch_tile_drain():
    """Walrus in this env rejects >1 sync-wait on the kernel-tail Drain
    (CTRL_NO_STRUCT setupSyncWait).  Split the waits across a chain of
    drain instructions, one wait each.  Idempotent, in-process only."""
    import concourse.tile as tile
    from concourse import mybir
    from concourse.vector_clock import ScopedClock

    if getattr(tile.TileContext._drain_and_barrier, "_split_patch", False):
        return

    def _split_drain(self, tick_clock, wait_clock):
        drain_inst = self.nc.sync.drain()
        wait_clock.add_sem_waits(
            drain_inst.ins, ScopedClock({None: tick_clock.global_clock}))
        si = drain_inst.ins.sync_info
        if si is not None and si.on_wait and len(si.on_wait) > 1:
            waits = list(si.on_wait)
            si.on_wait = waits[:1]
            for w in waits[1:]:
                d2 = self.nc.sync.drain()
                d2.ins.sync_info = mybir.SyncInfo(on_wait=[w], on_update=[])
        self.nc.all_engine_barrier()
        popped = self.nc._tile_sem_poison_stack.pop()
        assert popped is self._sem_poison
        self.nc.clear_and_free_semaphores(list(self.sems.allocated().values()))
        self.nc.all_engine_barrier()

    _split_drain._split_patch = True
    tile.TileContext._drain_and_barrier = _split_drain


def _split_waits(nc, max_waits=1):
    """This walrus build rejects instructions carrying more than one sync
    wait ('Too many sync wait commands' / 'ISA wrong length').  Move extra
    waits onto same-engine NoOps inserted immediately before."""
    from concourse import mybir

    n = 0
    for fn in nc.m.functions:
        for blk in fn.blocks:
            out = []
            for inst in blk.instructions:
                si = inst.sync_info
                if si is not None and si.on_wait and len(si.on_wait) > max_waits:
                    waits = list(si.on_wait)
                    for w in waits[:-max_waits]:
                        n += 1
                        nop = mybir.InstNoOp(
                            name=f"I-wsplit-{n}", ins=[], outs=[])
                        nop.engine = inst.engine
                        nop.sync_info = mybir.SyncInfo(
                            on_wait=[w], on_update=[])
                        out.append(nop)
                    si.on_wait = waits[-max_waits:]
                out.append(inst)
            blk.instructions = out
    return nc


def _build_nc():
    import concourse.bass as bass
    import concourse.tile as tile
    from concourse import mybir

    dt = mybir.dt
    AF = mybir.ActivationFunctionType
    ALU = mybir.AluOpType
    AX = mybir.AxisListType
    from concourse.masks import make_identity

    f32 = dt.float32
    bf16 = dt.bfloat16

    _patch_tile_drain()
    nc = bass.Bass()

    xs = nc.declare_dram_parameter("xs", [BPC, C, N], bf16, isOutput=False)
    wqk_t = nc.declare_dram_parameter("wqk_t", [C, 2 * C], bf16, isOutput=False)
    u_qk = nc.declare_dram_parameter("u_qk", [1, 2 * C], bf16, isOutput=False)
    wv = nc.declare_dram_parameter("wv", [CH, NH, C], bf16, isOutput=False)
    wpj48 = nc.declare_dram_parameter("wpj48", [CH, NH, C], bf16, isOutput=False)
    w1_t = nc.declare_dram_parameter("w1_t", [C, HID], bf16, isOutput=False)
    w2_t = nc.declare_dram_parameter("w2_t", [HID, C], bf16, isOutput=False)
    scale_row = nc.declare_dram_parameter("scale_row", [1, NH], f32, isOutput=False)
    out_d = nc.declare_dram_parameter("out", [BPC, C, N], bf16, isOutput=True)

    with tile.TileContext(nc) as tc:
        with (
            tc.tile_pool(name="consts", bufs=1) as consts,
            tc.tile_pool(name="ximg", bufs=2) as xpool,
            tc.tile_pool(name="qkp", bufs=3) as qkpool,
            tc.tile_pool(name="attn", bufs=2) as apool,
            tc.tile_pool(name="scr", bufs=3) as scr,
            tc.tile_pool(name="bwork", bufs=2) as bw,
            tc.tile_pool(name="pb", bufs=6, space="PSUM") as ps,
            tc.tile_pool(name="acc", bufs=1, space="PSUM") as psacc,
        ):
            def bcast_read(dst, dram_row, parts):
                src = bass.AP(
                    tensor=dram_row.tensor, offset=dram_row.offset,
                    ap=[[0, parts]] + [list(d) for d in dram_row.ap[-1:]])
                nc.gpsimd.dma_start(dst, src)

            # ---------------- constants ----------------
            wqk_sb = consts.tile([P, KS, 2 * C], bf16, tag="wqk")
            nc.gpsimd.dma_start(wqk_sb[:], wqk_t.rearrange("(s p) f -> p s f", p=P))
            wv_sb = consts.tile([CH, NH, C], bf16, tag="wv")
            nc.gpsimd.dma_start(wv_sb[:], wv[:])
            wpj_sb = consts.tile([CH, NH, C], bf16, tag="wpj")
            nc.gpsimd.dma_start(wpj_sb[:], wpj48[:])
            w1_sb = consts.tile([P, KS, HID], bf16, tag="w1")
            nc.gpsimd.dma_start(w1_sb[:], w1_t.rearrange("(s p) f -> p s f", p=P))
            w2_sb = consts.tile([P, KH, C], bf16, tag="w2")
            nc.gpsimd.dma_start(w2_sb[:], w2_t.rearrange("(s p) f -> p s f", p=P))
            uqk_sb = consts.tile([1, 2 * C], bf16, tag="uqk")
            nc.gpsimd.dma_start(uqk_sb[:], u_qk[:])
            ones_col = consts.tile([P, 1], bf16, tag="onescol")
            nc.vector.memset(ones_col[:], 1.0)
            ones_row = consts.tile([1, P], bf16, tag="onesrow")
            nc.vector.memset(ones_row[:], 1.0)
            identb = consts.tile([P, P], bf16, tag="identb")
            make_identity(nc, identb[:])
            schb = consts.tile([CH, NH], f32, tag="schb")
            bcast_read(schb[:], scale_row[0, :], parts=CH)

            xs_r = xs.rearrange("b (s p) n -> b p s n", p=P)
            out_r = out_d.rearrange("b (s p) n -> b p s n", p=P)

            # ------------- load both images upfront -------------
            x_tiles, rowpairs = [], []
            for img in range(BPC):
                x_sb = xpool.tile([P, KS, N], bf16, tag="x")
                for i in range(8):
                    sl = slice(i * 512, (i + 1) * 512)
                    nc.sync.dma_start(x_sb[:, :, sl], xs_r[img][:, :, sl])
                x_tiles.append(x_sb)
                # LN1 per-pixel rows: -mean and rstd (partition 0)
                mrow = xpool.tile([1, N], bf16, tag="mrow")
                rrow = xpool.tile([1, N], bf16, tag="rrow")
                rowpairs.append((mrow, rrow))

            def alloc_acc():
                acc1 = psacc.tile([CH, 400], f32, tag="acc1")  # S | q-norms²
                acc2 = psacc.tile([1, C], f32, tag="acc2")     # k-norms² row
                return acc1, acc2

            def phase_a(img, acc, interleave=()):
                """LN1 stats + qkT + S/norm accumulation for one image.
                interleave: emission closures spread into early chunks."""
                x_sb = x_tiles[img]
                mrow, rrow = rowpairs[img]
                acc1, acc2 = acc
                pend = None
                for t in range(NT):
                    if 1 <= t <= len(interleave):
                        interleave[t - 1]()
                    sl = slice(t * P, (t + 1) * P)
                    # x² for variance
                    xsq = scr.tile([P, KS, P], bf16, tag="xsq")
                    nc.vector.tensor_mul(xsq[:], x_sb[:, :, sl], x_sb[:, :, sl])
                    # column-form stats (per-pixel partitions), F=1 matmuls
                    pstat = ps.tile([P, 2], f32, tag="pb")
                    for s in range(KS):
                        nc.tensor.matmul(
                            pstat[:, 0:1], x_sb[:, s, sl], ones_col[:],
                            start=(s == 0), stop=(s == KS - 1))
                    for s in range(KS):
                        nc.tensor.matmul(
                            pstat[:, 1:2], xsq[:, s, :], ones_col[:],
                            start=(s == 0), stop=(s == KS - 1))
                    # stats eviction: mneg=-sum/C (bf16), rstd (f32 col)
                    stat2 = scr.tile([P, 33], bf16, tag="stat2")
                    vcol = scr.tile([P, 1], f32, tag="vcol")
                    msq = scr.tile([P, 1], f32, tag="msq")
                    rcol = scr.tile([P, 1], f32, tag="rcol")
                    nc.scalar.activation(
                        stat2[:, 0:1], pstat[:, 0:1], AF.Copy, scale=-1.0 / C)
                    nc.vector.tensor_scalar(
                        vcol[:], pstat[:, 1:2], 1.0 / C, EPS_LN,
                        op0=ALU.mult, op1=ALU.add)
                    nc.scalar.activation(msq[:], stat2[:, 0:1], AF.Square)
                    nc.vector.tensor_sub(vcol[:], vcol[:], msq[:])
                    nc.scalar.activation(rcol[:], vcol[:], AF.Sqrt)
                    nc.vector.reciprocal(rcol[:], rcol[:])
                    nc.vector.tensor_copy(stat2[:, 32:33], rcol[:])
                    # qkT x-part into PSUM (two banks)
                    pa1 = ps.tile([P, 512], f32, tag="pb")
                    pa2 = ps.tile([P, 256], f32, tag="pb")
                    for s in range(KS):
                        nc.tensor.matmul(
                            pa1[:], x_sb[:, s, sl], wqk_sb[:, s, 0:512],
                            start=(s == 0), stop=False)
                    for s in range(KS):
                        nc.tensor.matmul(
                            pa2[:], x_sb[:, s, sl], wqk_sb[:, s, 512:768],
                            start=(s == 0), stop=False)
                    # deferred S/norm accumulation from previous chunk
                    if pend is not None:
                        _emit_s(acc1, acc2, *pend)
                    # transpose stats to row form; rank-1 mean completion
                    ptr = ps.tile([33, P], bf16, tag="pb")
                    nc.tensor.transpose(ptr[:], stat2[:], identb[:])
                    nc.scalar.copy(mrow[0:1, sl], ptr[0:1, :])
                    nc.scalar.copy(rrow[0:1, sl], ptr[32:33, :])
                    nc.tensor.matmul(
                        pa1[:], mrow[0:1, sl], uqk_sb[0:1, 0:512],
                        start=False, stop=True)
                    nc.tensor.matmul(
                        pa2[:], mrow[0:1, sl], uqk_sb[0:1, 512:768],
                        start=False, stop=True)
                    # evictions: qk = rstd*pa (DVE), qksq = qk² (DVE bf16)
                    qk = qkpool.tile([P, 2 * C], bf16, tag="qk")
                    qksq = qkpool.tile([P, 2 * C], bf16, tag="qksq")
                    nc.vector.tensor_scalar_mul(qk[:, 0:512], pa1[:], rcol[:])
                    nc.vector.tensor_scalar_mul(qk[:, 512:768], pa2[:], rcol[:])
                    nc.vector.tensor_mul(qksq[:], qk[:], qk[:])
                    pend = (qk, qksq, t)
                _emit_s(acc1, acc2, *pend)

            def _emit_s(acc1, acc2, qk, qksq, t):
                st, sp = (t == 0), (t == NT - 1)
                for h in range(NH):
                    o = h * 2 * CH
                    nc.tensor.matmul(
                        acc1[:, h * CH:(h + 1) * CH],
                        qk[:, o:o + CH], qk[:, o + CH:o + 2 * CH],
                        start=st, stop=sp)
                for h in range(NH):
                    o = h * 2 * CH
                    nc.tensor.matmul(
                        acc1[:, 384 + h:385 + h],
                        qksq[:, o:o + CH], ones_col[:],
                        start=st, stop=sp)
                ksq = qksq.rearrange("p (h two c) -> p h two c", two=2, c=CH)
                nc.tensor.matmul(
                    acc2[:], ones_col[:], ksq[:, :, 1, :], start=st, stop=sp)

            def attn_stages(img, acc1, acc2):
                """Softmax + G build as a list of emission closures."""
                st = {}

                def s0():  # norms + scaled S + softmax -> sSb (bf16)
                    rq = apool.tile([CH, NH], f32, tag="rq")
                    nc.scalar.activation(rq[:], acc1[:, 384:392], AF.Sqrt)
                    nc.vector.tensor_scalar_max(rq[:], rq[:], EPS_NORM)
                    nc.vector.reciprocal(rq[:], rq[:])
                    nc.vector.tensor_mul(rq[:], rq[:], schb[:])
                    rk = apool.tile([1, C], f32, tag="rk")
                    nc.scalar.activation(rk[:], acc2[:], AF.Sqrt)
                    nc.vector.tensor_scalar_max(rk[:], rk[:], EPS_NORM)
                    nc.vector.reciprocal(rk[:], rk[:])
                    rkb = apool.tile([1, C], bf16, tag="rkb")
                    nc.vector.tensor_copy(rkb[:], rk[:])
                    rkb_ps = ps.tile([CH, C], f32, tag="pb")
                    nc.tensor.matmul(
                        rkb_ps[:], ones_row[0:1, 0:CH], rkb[0:1, :],
                        start=True, stop=True)
                    sS = apool.tile([CH, NH, CH], f32, tag="sS")
                    s_v = acc1[:, 0:384].rearrange("p (h e) -> p h e", e=CH)
                    nc.vector.tensor_mul(
                        sS[:], s_v, rq[:, :, None].to_broadcast((CH, NH, CH)))
                    rkb_v = rkb_ps.rearrange("p (h e) -> p h e", e=CH)
                    nc.vector.tensor_mul(sS[:], sS[:], rkb_v)
                    mx = apool.tile([CH, NH], f32, tag="mx")
                    nc.vector.reduce_max(mx[:], sS[:], axis=AX.X)
                    nc.vector.tensor_sub(
                        sS[:], sS[:], mx[:, :, None].to_broadcast((CH, NH, CH)))
                    nc.scalar.activation(sS[:], sS[:], AF.Exp)
                    esum = apool.tile([CH, NH], f32, tag="esum")
                    nc.vector.reduce_sum(esum[:], sS[:], axis=AX.X)
                    nc.vector.reciprocal(esum[:], esum[:])
                    sSb = apool.tile([CH, NH, CH], bf16, tag="sSb")
                    nc.vector.tensor_mul(
                        sSb[:], sS[:],
                        esum[:, :, None].to_broadcast((CH, NH, CH)))
                    st["sSb"] = sSb

                def s1():  # transpose attn per head
                    pt8 = ps.tile([CH, NH, CH], bf16, tag="pb")
                    for h in range(NH):
                        nc.tensor.transpose(
                            pt8[:, h, :], st["sSb"][:, h, :], identb[0:CH, 0:CH])
                    atT = apool.tile([CH, NH, CH], bf16, tag="atT")
                    nc.vector.tensor_copy(atT[:], pt8[:])
                    st["atT"] = atT

                def s2():  # awv_h = attn_h @ Wv_h
                    awv = apool.tile([CH, NH, C], bf16, tag="awv")
                    for h in range(NH):
                        paw = ps.tile([CH, C], f32, tag="pb")
                        nc.tensor.matmul(
                            paw[:], st["atT"][:, h, :], wv_sb[:, h, :],
                            start=True, stop=True)
                        if h % 2 == 0:
                            nc.vector.tensor_copy(awv[:, h, :], paw[:])
                        else:
                            nc.scalar.copy(awv[:, h, :], paw[:])
                    st["awv"] = awv

                def s3():  # G^T
                    gt_sb = apool.tile([P, KS, C], bf16, tag="gt")
                    for j in range(KS):
                        pgt = ps.tile([P, C], f32, tag="pb")
                        for h in range(NH):
                            nc.tensor.matmul(
                                pgt[:], st["awv"][:, h, j * P:(j + 1) * P],
                                wpj_sb[:, h, :], start=(h == 0), stop=(h == NH - 1))
                        if j % 2 == 0:
                            nc.vector.tensor_copy(gt_sb[:, j, :], pgt[:])
                        else:
                            nc.scalar.copy(gt_sb[:, j, :], pgt[:])
                    st["gt"] = gt_sb

                def s4():  # uG row
                    pug = ps.tile([1, C], f32, tag="pb")
                    for s in range(KS):
                        nc.tensor.matmul(
                            pug[:], ones_col[:], st["gt"][:, s, :],
                            start=(s == 0), stop=(s == KS - 1))
                    ug = apool.tile([1, C], bf16, tag="ug")
                    nc.vector.tensor_copy(ug[:], pug[:])
                    st["ug"] = ug

                return [s0, s1, s2, s3, s4], st

            def phase_b_chunk(img, st, f, carry):
                """One 256-pixel chunk of the attn-apply + FFN pass.
                carry holds deferred ffn emission state from chunk f-1."""
                x_sb = x_tiles[img]
                mrow, rrow = rowpairs[img]
                sl = slice(f * FG, (f + 1) * FG)
                # rstd broadcast for this chunk
                bc1 = ps.tile([P, FG], f32, tag="pb")
                nc.tensor.matmul(
                    bc1[:], ones_row[0:1, :], rrow[0:1, sl],
                    start=True, stop=True)
                rb = scr.tile([P, FG], bf16, tag="rb")
                nc.scalar.copy(rb[:], bc1[:])
                # G pass + rank-1 mean term
                pgA = ps.tile([P, 2, FG], f32, tag="pb")
                pgB = ps.tile([P, FG], f32, tag="pb")
                gt, ug = st["gt"], st["ug"]
                for j in range(KS):
                    dst = pgA[:, j, :] if j < 2 else pgB[:]
                    for s in range(KS):
                        nc.tensor.matmul(
                            dst, gt[:, s, j * P:(j + 1) * P], x_sb[:, s, sl],
                            start=(s == 0), stop=False)
                    nc.tensor.matmul(
                        dst, ug[0:1, j * P:(j + 1) * P], mrow[0:1, sl],
                        start=False, stop=True)
                # deferred ffn from previous chunk fills the PE here
                if carry is not None:
                    _emit_ffn(img, *carry)
                # y = x + rstd*(G-branch); stats inputs
                y = bw.tile([P, KS, FG], bf16, tag="y")
                ab = bw.tile([P, KS, FG], bf16, tag="ab")
                rb_bc2 = rb[:, None, :].to_broadcast((P, 2, FG))
                nc.vector.tensor_mul(ab[:, 0:2, :], pgA[:], rb_bc2)
                nc.vector.tensor_mul(ab[:, 2, :], pgB[:], rb[:])
                nc.vector.tensor_add(y[:], x_sb[:, :, sl], ab[:])
                ysq = bw.tile([P, KS, FG], bf16, tag="ysq")
                nc.vector.tensor_mul(ysq[:], y[:], y[:])
                # LN2 column stats per 128-px half
                pstat2 = ps.tile([P, 2, 2], f32, tag="pb")
                for half in range(2):
                    hsl = slice(half * P, (half + 1) * P)
                    for s in range(KS):
                        nc.tensor.matmul(
                            pstat2[:, half, 0:1], y[:, s, hsl], ones_col[:],
                            start=(s == 0), stop=(s == KS - 1))
                    for s in range(KS):
                        nc.tensor.matmul(
                            pstat2[:, half, 1:2], ysq[:, s, hsl], ones_col[:],
                            start=(s == 0), stop=(s == KS - 1))
                stat22 = scr.tile([P, 2, 33], bf16, tag="stat22")
                vcol2 = scr.tile([P, 2], f32, tag="vcol2")
                msq2 = scr.tile([P, 2], f32, tag="msq2")
                rcol2 = scr.tile([P, 2], f32, tag="rcol2")
                nc.scalar.activation(
                    stat22[:, :, 0], pstat2[:, :, 0], AF.Copy, scale=-1.0 / C)
                nc.vector.tensor_scalar(
                    vcol2[:], pstat2[:, :, 1], 1.0 / C, EPS_LN,
                    op0=ALU.mult, op1=ALU.add)
                nc.scalar.activation(msq2[:], stat22[:, :, 0], AF.Square)
                nc.vector.tensor_sub(vcol2[:], vcol2[:], msq2[:])
                nc.scalar.activation(rcol2[:], vcol2[:], AF.Sqrt)
                nc.vector.reciprocal(rcol2[:], rcol2[:])
                nc.vector.tensor_copy(stat22[:, :, 32], rcol2[:])
                # rows4: [m_h0 | r_h0 | m_h1 | r_h1]
                ptr2 = ps.tile([33, 2, P], bf16, tag="pb")
                for half in range(2):
                    nc.tensor.transpose(
                        ptr2[:, half, :], stat22[:, half, :], identb[:])
                rows4 = scr.tile([1, 4, P], bf16, tag="rows4")
                for half in range(2):
                    nc.scalar.copy(rows4[0:1, 2 * half, :], ptr2[0:1, half, :])
                    nc.scalar.copy(
                        rows4[0:1, 2 * half + 1, :], ptr2[32:33, half, :])
                bc2 = ps.tile([P, 2, 2, P], f32, tag="pb")
                for half in range(2):
                    for mr in range(2):
                        nc.tensor.matmul(
                            bc2[:, half, mr, :], ones_row[0:1, :],
                            rows4[0:1, 2 * half + mr, :],
                            start=True, stop=True)
                bcs = scr.tile([P, 2, 2, P], bf16, tag="bcs")
                nc.scalar.copy(bcs[:], bc2[:])
                yn = bw.tile([P, KS, FG], bf16, tag="yn")
                t3 = bw.tile([P, KS, FG], bf16, tag="t3")
                for half in range(2):
                    hsl = slice(half * P, (half + 1) * P)
                    nc.vector.tensor_add(
                        t3[:, :, hsl], y[:, :, hsl],
                        bcs[:, half, 0, None, :].to_broadcast((P, KS, P)))
                    nc.vector.tensor_mul(
                        yn[:, :, hsl], t3[:, :, hsl],
                        bcs[:, half, 1, None, :].to_broadcast((P, KS, P)))
                return (f, y, yn)

            def _emit_ffn(img, f, y, yn):
                sl = slice(f * FG, (f + 1) * FG)
                h_sb = bw.tile([P, KH, FG], bf16, tag="h")
                for g in range(6):
                    ph = ps.tile([P, 2, FG], f32, tag="pb")
                    for m2 in range(2):
                        mh = g * 2 + m2
                        for s in range(KS):
                            nc.tensor.matmul(
                                ph[:, m2, :],
                                w1_sb[:, s, mh * P:(mh + 1) * P], yn[:, s, :],
                                start=(s == 0), stop=(s == KS - 1))
                    nc.scalar.activation(
                        h_sb[:, g * 2:(g + 1) * 2, :], ph[:], AF.Gelu)
                poA = ps.tile([P, 2, FG], f32, tag="pb")
                poB = ps.tile([P, FG], f32, tag="pb")
                for mo in range(KS):
                    dst = poA[:, mo, :] if mo < 2 else poB[:]
                    for s in range(KH):
                        nc.tensor.matmul(
                            dst, w2_sb[:, s, mo * P:(mo + 1) * P], h_sb[:, s, :],
                            start=(s == 0), stop=(s == KH - 1))
                o_sb = bw.tile([P, KS, FG], bf16, tag="o")
                nc.vector.tensor_add(o_sb[:, 0:2, :], poA[:], y[:, 0:2, :])
                nc.vector.tensor_add(o_sb[:, 2, :], poB[:], y[:, 2, :])
                nc.sync.dma_start(out_r[img][:, :, sl], o_sb[:])

            # ----------------- schedule -----------------
            acc0 = alloc_acc()
            phase_a(0, acc0)
            stages0, st0 = attn_stages(0, *acc0)
            accB = alloc_acc()
            phase_a(1, accB, interleave=stages0)
            stages1, st1 = attn_stages(1, *accB)
            # phase B image 0, attn stages of image 1 interleaved
            carry = None
            for f in range(NFG):
                if f < len(stages1):
                    stages1[f]()
                carry = phase_b_chunk(0, st0, f, carry)
            _emit_ffn(0, *carry)
            carry = None
            for f in range(NFG):
                carry = phase_b_chunk(1, st1, f, carry)
            _emit_ffn(1, *carry)

    return _split_waits(nc)


def _prep_weights(inputs):
    import ml_dtypes
    bf = ml_dtypes.bfloat16
    w_qkv = np.asarray(inputs["w_qkv"], np.float32)
    g1 = np.asarray(inputs["g1"], np.float32)
    g2 = np.asarray(inputs["g2"], np.float32)
    for name in ("beta1", "beta2", "b_qkv", "b_proj", "b_ffn1", "b_ffn2"):
        assert not np.any(np.asarray(inputs[name])), f"{name} nonzero unsupported"
    wg = w_qkv * g1[None, :]  # fold LN gamma into qkv weights
    wg3 = wg.reshape(NH, 3 * CH, C)
    wq = wg3[:, 0:CH, :]
    wk = wg3[:, CH:2 * CH, :]
    wv_ = wg3[:, 2 * CH:3 * CH, :]
    # qk columns interleaved per head: j = h*96 + (0..47 q | 48..95 k)
    wqk = np.concatenate([wq, wk], axis=1).reshape(2 * C, C)
    wqk_t = np.ascontiguousarray(wqk.T)  # [384, 768]
    u_qk = wqk.sum(axis=1)[None, :]  # [1, 768]
    wv_t = np.ascontiguousarray(wv_.transpose(1, 0, 2))  # [48, NH, 384]
    wpj48 = np.ascontiguousarray(
        np.asarray(inputs["w_proj"], np.float32).T.reshape(NH, CH, C)
        .transpose(1, 0, 2))
    w1g = np.asarray(inputs["w_ffn1"], np.float32) * g2[None, :]
    w1_t = np.ascontiguousarray(w1g.T)  # [384, 1536]
    w2_t = np.ascontiguousarray(np.asarray(inputs["w_ffn2"], np.float32).T)
    ls = np.asarray(inputs["logit_scale"], np.float32).reshape(NH)
    scale_row = np.exp(np.minimum(ls, LOGIT_MAX))[None, :]
    return dict(
        wqk_t=wqk_t.astype(bf), u_qk=np.ascontiguousarray(u_qk).astype(bf),
        wv=wv_t.astype(bf), wpj48=wpj48.astype(bf),
        w1_t=w1_t.astype(bf), w2_t=w2_t.astype(bf),
        scale_row=np.ascontiguousarray(scale_row).astype(np.float32))


def _make_in_maps(inputs):
    import ml_dtypes
    x = np.asarray(inputs["x"], np.float32).reshape(B, C, N).astype(
        ml_dtypes.bfloat16)
    wmap = _prep_weights(inputs)
    in_maps = []
    for c in range(NCORES):
        m = dict(wmap)
        m["xs"] = np.ascontiguousarray(x[c * BPC:(c + 1) * BPC])
        in_maps.append(m)
    return in_maps


def kernel(**inputs):
    from concourse.bass_utils import run_bass_kernel_spmd

    if "nc" not in _CACHE:
        _CACHE["nc"] = _build_nc()
    nc = _CACHE["nc"]
    in_maps = _make_in_maps(inputs)
    res = run_bass_kernel_spmd(nc, in_maps, list(range(NCORES)))
    out = np.concatenate(
        [np.asarray(r["out"], np.float32) for r in res.results], axis=0)
    return out.reshape(B, C, 64, 64)


# revision 16
# speedup vs baseline: 1.6600x; 1.5524x over previous
"""Trainium2 Bass kernel for nn_CATransformer1 (XCiT-style channel-attention block).

Sharding: data-parallel over batch. 16 images / 8 cores = 2 images per core.
Weights replicated; no collectives.

V2 design (bf16 everywhere):
  - x is DMAed once per image (bf16) and stays SBUF-resident across both
    passes; output written back as bf16 and widened on host.
  - All matmuls run in bf16 (full rate at any free-dim size on TRN2).
  - LN1/LN2 stats are computed in column form (per-pixel partitions) with
    F=1 ones-matmuls (nearly free on the PE), then transposed to row form
    for the rank-1 mean terms and row-broadcasts.
  - LN1 mean is folded into the QKV matmul as a rank-1 K=1 accumulation
    (lhsT=mneg row, rhs=u row); rstd applied at PSUM eviction via
    per-partition tensor_scalar (pixels are partitions).
  - Attention output + projection collapsed into per-image G = Wproj @
    concat_h(attn_h @ Wv_h); attn branch = rstd * (G@x - m*uG) via the same
    rank-1 trick; LN2 materializes yn (bf16) for the FFN.
  - Eviction work split across DVE and Activation engines; emission is
    software-pipelined (S-accum deferred one chunk; image-1 attention block
    interleaved into image-0 phase B) so the PE stays fed.
"""

import numpy as np

B, C, NH, CH, N, HID = 16, 384, 8, 48, 4096, 1536
NCORES = 8
BPC = B // NCORES  # images per core
P = 128
KS = C // P   # 3 k-subtiles for C
KH = HID // P  # 12 k-subtiles for HID
NT = N // P   # 32 pixel chunks (phase A)
FG = 512      # phase B pixel chunk
NFG = N // FG
LOGIT_MAX = float(np.log(1.0 / 0.01))
EPS_LN = 1e-5
EPS_NORM = 1e-12

_CACHE = {}


def _patch_tile_drain():
    """Walrus in this env rejects >1 sync-wait on the kernel-tail Drain
    (CTRL_NO_STRUCT setupSyncWait).  Split the waits across a chain of
    drain instructions, one wait each.  Idempotent, in-process only."""
    import concourse.tile as tile
    from concourse import mybir
    from concourse.vector_clock import ScopedClock

    if getattr(tile.TileContext._drain_and_barrier, "_split_patch", False):
        return

    def _split_drain(self, tick_clock, wait_clock):
        drain_inst = self.nc.sync.drain()
        wait_clock.add_sem_waits(
            drain_inst.ins, ScopedClock({None: tick_clock.global_clock}))
        si = drain_inst.ins.sync_info
        if si is not None and si.on_wait and len(si.on_wait) > 1:
            waits = list(si.on_wait)
            si.on_wait = waits[:1]
            for w in waits[1:]:
                d2 = self.nc.sync.drain()
                d2.ins.sync_info = mybir.SyncInfo(on_wait=[w], on_update=[])
        self.nc.all_engine_barrier()
        popped = self.nc._tile_sem_poison_stack.pop()
        assert popped is self._sem_poison
        self.nc.clear_and_free_semaphores(list(self.sems.allocated().values()))
        self.nc.all_engine_barrier()

    _split_drain._split_patch = True
    tile.TileContext._drain_and_barrier = _split_drain


def _split_waits(nc, max_waits=1):
    """This walrus build rejects instructions carrying more than one sync
    wait ('Too many sync wait commands' / 'ISA wrong length').  Move extra
    waits onto same-engine NoOps inserted immediately before."""
    from concourse import mybir

    n = 0
    for fn in nc.m.functions:
        for blk in fn.blocks:
            out = []
            for inst in blk.instructions:
                si = inst.sync_info
                if si is not None and si.on_wait and len(si.on_wait) > max_waits:
                    waits = list(si.on_wait)
                    for w in waits[:-max_waits]:
                        n += 1
                        nop = mybir.InstNoOp(
                            name=f"I-wsplit-{n}", ins=[], outs=[])
                        nop.engine = inst.engine
                        nop.sync_info = mybir.SyncInfo(
                            on_wait=[w], on_update=[])
                        out.append(nop)
                    si.on_wait = waits[-max_waits:]
                out.append(inst)
            blk.instructions = out
    return nc


def _build_nc():
    import concourse.bass as bass
    import concourse.tile as tile
    from concourse import mybir

    dt = mybir.dt
    AF = mybir.ActivationFunctionType
    ALU = mybir.AluOpType
    AX = mybir.AxisListType
    from concourse.masks import make_identity

    f32 = dt.float32
    bf16 = dt.bfloat16

    _patch_tile_drain()
    nc = bass.Bass()

    xs = nc.declare_dram_parameter("xs", [BPC, C, N], bf16, isOutput=False)
    wqk_t = nc.declare_dram_parameter("wqk_t", [C, 2 * C], bf16, isOutput=False)
    u_qk = nc.declare_dram_parameter("u_qk", [1, 2 * C], bf16, isOutput=False)
    wv = nc.declare_dram_parameter("wv", [CH, NH, C], bf16, isOutput=False)
    wpj48 = nc.declare_dram_parameter("wpj48", [CH, NH, C], bf16, isOutput=False)
    f8 = dt.float8e4
    w1_t = nc.declare_dram_parameter("w1_t", [C, HID], f8, isOutput=False)
    w2_t = nc.declare_dram_parameter("w2_t", [HID, C], f8, isOutput=False)
    scale_row = nc.declare_dram_parameter("scale_row", [1, NH], f32, isOutput=False)
    out_d = nc.declare_dram_parameter("out", [BPC, C, N], bf16, isOutput=True)

    with tile.TileContext(nc) as tc:
        with (
            tc.tile_pool(name="consts", bufs=1) as consts,
            tc.tile_pool(name="ximg", bufs=2) as xpool,
            tc.tile_pool(name="qkp", bufs=2) as qkpool,
            tc.tile_pool(name="attn", bufs=2) as apool,
            tc.tile_pool(name="scr", bufs=3) as scr,
            tc.tile_pool(name="bwork", bufs=2) as bw,
            tc.tile_pool(name="pb", bufs=6, space="PSUM") as ps,
            tc.tile_pool(name="acc", bufs=1, space="PSUM") as psacc,
        ):
            def bcast_read(dst, dram_row, parts):
                src = bass.AP(
                    tensor=dram_row.tensor, offset=dram_row.offset,
                    ap=[[0, parts]] + [list(d) for d in dram_row.ap[-1:]])
                nc.gpsimd.dma_start(dst, src)

            # ---------------- constants ----------------
            wqk_sb = consts.tile([P, KS, 2 * C], bf16, tag="wqk")
            nc.gpsimd.dma_start(wqk_sb[:], wqk_t.rearrange("(s p) f -> p s f", p=P))
            wv_sb = consts.tile([CH, NH, C], bf16, tag="wv")
            nc.gpsimd.dma_start(wv_sb[:], wv[:])
            wpj_sb = consts.tile([CH, NH, C], bf16, tag="wpj")
            nc.gpsimd.dma_start(wpj_sb[:], wpj48[:])
            w1_sb = consts.tile([P, KS, HID], f8, tag="w1")
            nc.gpsimd.dma_start(w1_sb[:], w1_t.rearrange("(s p) f -> p s f", p=P))
            w2_sb = consts.tile([P, KH, C], f8, tag="w2")
            nc.gpsimd.dma_start(w2_sb[:], w2_t.rearrange("(s p) f -> p s f", p=P))
            uqk_sb = consts.tile([1, 2 * C], bf16, tag="uqk")
            nc.gpsimd.dma_start(uqk_sb[:], u_qk[:])
            ones_col = consts.tile([P, 1], bf16, tag="onescol")
            nc.vector.memset(ones_col[:], 1.0)
            ones_row = consts.tile([1, P], bf16, tag="onesrow")
            nc.vector.memset(ones_row[:], 1.0)
            identb = consts.tile([P, P], bf16, tag="identb")
            make_identity(nc, identb[:])
            schb = consts.tile([CH, NH], f32, tag="schb")
            bcast_read(schb[:], scale_row[0, :], parts=CH)

            xs_r = xs.rearrange("b (s p) n -> b p s n", p=P)
            out_r = out_d.rearrange("b (s p) n -> b p s n", p=P)

            # ------------- load both images upfront -------------
            x_tiles, rowpairs = [], []
            for img in range(BPC):
                x_sb = xpool.tile([P, KS, N], bf16, tag="x")
                for i in range(8):
                    sl = slice(i * 512, (i + 1) * 512)
                    nc.sync.dma_start(x_sb[:, :, sl], xs_r[img][:, :, sl])
                x_tiles.append(x_sb)
                # LN1 per-pixel rows: -mean and rstd (partition 0)
                mrow = xpool.tile([1, N], bf16, tag="mrow")
                rrow = xpool.tile([1, N], bf16, tag="rrow")
                rowpairs.append((mrow, rrow))

            def alloc_acc():
                acc1 = psacc.tile([CH, 400], f32, tag="acc1")  # S | q-norms²
                acc2 = psacc.tile([1, C], f32, tag="acc2")     # k-norms² row
                return acc1, acc2

            def phase_a(img, acc, interleave=()):
                """LN1 stats + qkT + S/norm accumulation for one image.
                interleave: emission closures spread into early chunks."""
                x_sb = x_tiles[img]
                mrow, rrow = rowpairs[img]
                acc1, acc2 = acc
                pend = None
                for t in range(NT):
                    if 1 <= t <= len(interleave):
                        interleave[t - 1]()
                    sl = slice(t * P, (t + 1) * P)
                    # x² for variance
                    xsq = scr.tile([P, KS, P], bf16, tag="xsq", bufs=2)
                    nc.vector.tensor_mul(xsq[:], x_sb[:, :, sl], x_sb[:, :, sl])
                    # column-form stats (per-pixel partitions), F=1 matmuls
                    pstat = ps.tile([P, 2], f32, tag="pb")
                    for s in range(KS):
                        nc.tensor.matmul(
                            pstat[:, 0:1], x_sb[:, s, sl], ones_col[:],
                            start=(s == 0), stop=(s == KS - 1))
                    for s in range(KS):
                        nc.tensor.matmul(
                            pstat[:, 1:2], xsq[:, s, :], ones_col[:],
                            start=(s == 0), stop=(s == KS - 1))
                    # stats eviction: mneg=-sum/C (bf16), rstd (f32 col)
                    stat2 = scr.tile([P, 33], bf16, tag="stat2")
                    vcol = scr.tile([P, 1], f32, tag="vcol")
                    msq = scr.tile([P, 1], f32, tag="msq")
                    rcol = scr.tile([P, 1], f32, tag="rcol")
                    nc.scalar.activation(
                        stat2[:, 0:1], pstat[:, 0:1], AF.Copy, scale=-1.0 / C)
                    nc.vector.tensor_scalar(
                        vcol[:], pstat[:, 1:2], 1.0 / C, EPS_LN,
                        op0=ALU.mult, op1=ALU.add)
                    nc.scalar.activation(msq[:], stat2[:, 0:1], AF.Square)
                    nc.vector.tensor_sub(vcol[:], vcol[:], msq[:])
                    nc.scalar.activation(rcol[:], vcol[:], AF.Sqrt)
                    nc.vector.reciprocal(rcol[:], rcol[:])
                    nc.vector.tensor_copy(stat2[:, 32:33], rcol[:])
                    # qkT x-part into PSUM (two banks)
                    pa1 = ps.tile([P, 512], f32, tag="pb")
                    pa2 = ps.tile([P, 256], f32, tag="pb")
                    for s in range(KS):
                        nc.tensor.matmul(
                            pa1[:], x_sb[:, s, sl], wqk_sb[:, s, 0:512],
                            start=(s == 0), stop=False)
                    for s in range(KS):
                        nc.tensor.matmul(
                            pa2[:], x_sb[:, s, sl], wqk_sb[:, s, 512:768],
                            start=(s == 0), stop=False)
                    # deferred S/norm accumulation from previous chunk
                    if pend is not None:
                        _emit_s(acc1, acc2, *pend)
                    # transpose stats to row form; rank-1 mean completion
                    ptr = ps.tile([33, P], bf16, tag="pb")
                    nc.tensor.transpose(ptr[:], stat2[:], identb[:])
                    nc.scalar.copy(mrow[0:1, sl], ptr[0:1, :])
                    nc.scalar.copy(rrow[0:1, sl], ptr[32:33, :])
                    nc.tensor.matmul(
                        pa1[:], mrow[0:1, sl], uqk_sb[0:1, 0:512],
                        start=False, stop=True)
                    nc.tensor.matmul(
                        pa2[:], mrow[0:1, sl], uqk_sb[0:1, 512:768],
                        start=False, stop=True)
                    # evictions: qk = rstd*pa (DVE), qksq = qk² (DVE bf16)
                    qk = qkpool.tile([P, 2 * C], bf16, tag="qk")
                    qksq = qkpool.tile([P, 2 * C], bf16, tag="qksq")
                    nc.vector.tensor_scalar_mul(qk[:, 0:512], pa1[:], rcol[:])
                    nc.vector.tensor_scalar_mul(qk[:, 512:768], pa2[:], rcol[:])
                    nc.vector.tensor_mul(qksq[:], qk[:], qk[:])
                    pend = (qk, qksq, t)
                _emit_s(acc1, acc2, *pend)

            def _emit_s(acc1, acc2, qk, qksq, t):
                st, sp = (t == 0), (t == NT - 1)
                for h in range(NH):
                    o = h * 2 * CH
                    nc.tensor.matmul(
                        acc1[:, h * CH:(h + 1) * CH],
                        qk[:, o:o + CH], qk[:, o + CH:o + 2 * CH],
                        start=st, stop=sp)
                for h in range(NH):
                    o = h * 2 * CH
                    nc.tensor.matmul(
                        acc1[:, 384 + h:385 + h],
                        qksq[:, o:o + CH], ones_col[:],
                        start=st, stop=sp)
                ksq = qksq.rearrange("p (h two c) -> p h two c", two=2, c=CH)
                nc.tensor.matmul(
                    acc2[:], ones_col[:], ksq[:, :, 1, :], start=st, stop=sp)

            def attn_stages(img, acc1, acc2):
                """Softmax + G build as a list of emission closures."""
                st = {}

                def s0():  # norms + scaled S + softmax -> sSb (bf16)
                    rq = apool.tile([CH, NH], f32, tag="rq", bufs=1)
                    nc.scalar.activation(rq[:], acc1[:, 384:392], AF.Sqrt)
                    nc.vector.tensor_scalar_max(rq[:], rq[:], EPS_NORM)
                    nc.vector.reciprocal(rq[:], rq[:])
                    nc.vector.tensor_mul(rq[:], rq[:], schb[:])
                    rk = apool.tile([1, C], f32, tag="rk", bufs=1)
                    nc.scalar.activation(rk[:], acc2[:], AF.Sqrt)
                    nc.vector.tensor_scalar_max(rk[:], rk[:], EPS_NORM)
                    nc.vector.reciprocal(rk[:], rk[:])
                    rkb = apool.tile([1, C], bf16, tag="rkb", bufs=1)
                    nc.vector.tensor_copy(rkb[:], rk[:])
                    rkb_ps = ps.tile([CH, C], f32, tag="pb")
                    nc.tensor.matmul(
                        rkb_ps[:], ones_row[0:1, 0:CH], rkb[0:1, :],
                        start=True, stop=True)
                    sS = apool.tile([CH, NH, CH], f32, tag="sS", bufs=1)
                    s_v = acc1[:, 0:384].rearrange("p (h e) -> p h e", e=CH)
                    nc.vector.tensor_mul(
                        sS[:], s_v, rq[:, :, None].to_broadcast((CH, NH, CH)))
                    rkb_v = rkb_ps.rearrange("p (h e) -> p h e", e=CH)
                    nc.vector.tensor_mul(sS[:], sS[:], rkb_v)
                    mx = apool.tile([CH, NH], f32, tag="mx", bufs=1)
                    nc.vector.reduce_max(mx[:], sS[:], axis=AX.X)
                    nc.vector.tensor_sub(
                        sS[:], sS[:], mx[:, :, None].to_broadcast((CH, NH, CH)))
                    nc.scalar.activation(sS[:], sS[:], AF.Exp)
                    esum = apool.tile([CH, NH], f32, tag="esum", bufs=1)
                    nc.vector.reduce_sum(esum[:], sS[:], axis=AX.X)
                    nc.vector.reciprocal(esum[:], esum[:])
                    sSb = apool.tile([CH, NH, CH], bf16, tag="sSb", bufs=1)
                    nc.vector.tensor_mul(
                        sSb[:], sS[:],
                        esum[:, :, None].to_broadcast((CH, NH, CH)))
                    st["sSb"] = sSb

                def s1():  # transpose attn per head
                    pt8 = ps.tile([CH, NH, CH], bf16, tag="pb")
                    for h in range(NH):
                        nc.tensor.transpose(
                            pt8[:, h, :], st["sSb"][:, h, :], identb[0:CH, 0:CH])
                    atT = apool.tile([CH, NH, CH], bf16, tag="atT", bufs=1)
                    nc.vector.tensor_copy(atT[:], pt8[:])
                    st["atT"] = atT

                def s2():  # awv_h = attn_h @ Wv_h
                    awv = apool.tile([CH, NH, C], bf16, tag="awv", bufs=1)
                    for h in range(NH):
                        paw = ps.tile([CH, C], f32, tag="pb")
                        nc.tensor.matmul(
                            paw[:], st["atT"][:, h, :], wv_sb[:, h, :],
                            start=True, stop=True)
                        if h % 2 == 0:
                            nc.vector.tensor_copy(awv[:, h, :], paw[:])
                        else:
                            nc.scalar.copy(awv[:, h, :], paw[:])
                    st["awv"] = awv

                def s3():  # G^T
                    gt_sb = apool.tile([P, KS, C], bf16, tag="gt")
                    for j in range(KS):
                        pgt = ps.tile([P, C], f32, tag="pb")
                        for h in range(NH):
                            nc.tensor.matmul(
                                pgt[:], st["awv"][:, h, j * P:(j + 1) * P],
                                wpj_sb[:, h, :], start=(h == 0), stop=(h == NH - 1))
                        if j % 2 == 0:
                            nc.vector.tensor_copy(gt_sb[:, j, :], pgt[:])
                        else:
                            nc.scalar.copy(gt_sb[:, j, :], pgt[:])
                    st["gt"] = gt_sb

                def s4():  # uG row
                    pug = ps.tile([1, C], f32, tag="pb")
                    for s in range(KS):
                        nc.tensor.matmul(
                            pug[:], ones_col[:], st["gt"][:, s, :],
                            start=(s == 0), stop=(s == KS - 1))
                    ug = apool.tile([1, C], bf16, tag="ug")
                    nc.vector.tensor_copy(ug[:], pug[:])
                    st["ug"] = ug

                return [s0, s1, s2, s3, s4], st

            # per-image y / yn tiles (yn in fp8 for the DR ffn)
            f8sc = 64.0  # host scales w1/w2 by 64 (fp8 e4m3 denormal floor)

            def phase_b1(img, st, interleave=()):
                """G-branch apply + residual + LN2; produces y (bf16) and
                yn (fp8) for the whole image."""
                mrow, rrow = rowpairs[img]
                # y reuses the x image slots (x residency ends with phase A;
                # B1 re-reads x chunk-wise from DRAM)
                y = xpool.tile([P, KS, N], bf16, tag="x", name=f"y{img}")
                yn = bw.tile([P, KS, N], f8, tag="yn", bufs=1)
                gt, ug = st["gt"], st["ug"]
                pend = None
                for f in range(NFG):
                    if f < len(interleave):
                        interleave[f]()
                    sl = slice(f * FG, (f + 1) * FG)
                    xb = scr.tile([P, KS, FG], bf16, tag="xb", bufs=2)
                    nc.sync.dma_start(xb[:], xs_r[img][:, :, sl])
                    bc1 = ps.tile([P, FG], f32, tag="pb")
                    nc.tensor.matmul(
                        bc1[:], ones_row[0:1, :], rrow[0:1, sl],
                        start=True, stop=True)
                    rb = scr.tile([P, FG], bf16, tag="rb", bufs=2)
                    nc.scalar.copy(rb[:], bc1[:])
                    pgs = []
                    for j in range(KS):
                        pg = ps.tile([P, FG], f32, tag="pb", name=f"pg{j}")
                        for s in range(KS):
                            nc.tensor.matmul(
                                pg[:], gt[:, s, j * P:(j + 1) * P],
                                xb[:, s, :], start=(s == 0), stop=False)
                        nc.tensor.matmul(
                            pg[:], ug[0:1, j * P:(j + 1) * P], mrow[0:1, sl],
                            start=False, stop=True)
                        pgs.append(pg)
                    ab = bw.tile([P, KS, FG], bf16, tag="ab", bufs=1)
                    for j in range(KS):
                        nc.vector.tensor_mul(ab[:, j, :], pgs[j][:], rb[:])
                    nc.vector.tensor_add(y[:, :, sl], xb[:], ab[:])
                    ysq = bw.tile([P, KS, FG], bf16, tag="ysq", bufs=2)
                    nc.vector.tensor_mul(ysq[:], y[:, :, sl], y[:, :, sl])
                    # LN2 row stats for this chunk (deferred one chunk so the
                    # PE stays on G matmuls while DVE/ACT chew the rows)
                    if pend is not None:
                        _emit_ln2(img, y, yn, *pend)
                    pend = (f, ysq)
                _emit_ln2(img, y, yn, *pend)
                return y, yn

            def _emit_ln2(img, y, yn, f, ysq):
                sl = slice(f * FG, (f + 1) * FG)
                p2a = ps.tile([1, FG], f32, tag="pb")
                p2b = ps.tile([1, FG], f32, tag="pb")
                for s in range(KS):
                    nc.tensor.matmul(
                        p2a[:], ones_col[:], y[:, s, sl],
                        start=(s == 0), stop=(s == KS - 1))
                for s in range(KS):
                    nc.tensor.matmul(
                        p2b[:], ones_col[:], ysq[:, s, :],
                        start=(s == 0), stop=(s == KS - 1))
                m2b = scr.tile([1, FG], bf16, tag="m2b", bufs=2)
                nc.scalar.activation(m2b[:], p2a[:], AF.Copy, scale=-1.0 / C)
                vrow = scr.tile([1, FG], f32, tag="vrow", bufs=2)
                nc.vector.tensor_scalar(
                    vrow[:], p2b[:], 1.0 / C, EPS_LN, op0=ALU.mult, op1=ALU.add)
                msq = scr.tile([1, FG], f32, tag="msqr", bufs=2)
                nc.scalar.activation(msq[:], m2b[:], AF.Square)
                nc.vector.tensor_sub(vrow[:], vrow[:], msq[:])
                srow = scr.tile([1, FG], f32, tag="srow", bufs=2)
                nc.scalar.activation(srow[:], vrow[:], AF.Sqrt)
                r2f = scr.tile([1, FG], f32, tag="r2f", bufs=2)
                nc.vector.reciprocal(r2f[:], srow[:])
                r2b = scr.tile([1, FG], bf16, tag="r2b", bufs=2)
                nc.vector.tensor_copy(r2b[:], r2f[:])
                bcm = ps.tile([P, FG], f32, tag="pb")
                nc.tensor.matmul(
                    bcm[:], ones_row[0:1, :], m2b[0:1, :], start=True, stop=True)
                bcr = ps.tile([P, FG], f32, tag="pb")
                nc.tensor.matmul(
                    bcr[:], ones_row[0:1, :], r2b[0:1, :], start=True, stop=True)
                mbc = scr.tile([P, FG], bf16, tag="mbc", bufs=2)
                nc.scalar.copy(mbc[:], bcm[:])
                rbc = scr.tile([P, FG], bf16, tag="rbc", bufs=2)
                nc.scalar.copy(rbc[:], bcr[:])
                t3 = bw.tile([P, KS, FG], bf16, tag="t3", bufs=1)
                nc.vector.tensor_add(
                    t3[:], y[:, :, sl], mbc[:, None, :].to_broadcast((P, KS, FG)))
                nc.vector.tensor_mul(
                    yn[:, :, sl], t3[:], rbc[:, None, :].to_broadcast((P, KS, FG)))

            def phase_b2(img, y, yn):
                """FFN in fp8 DoubleRow + residual + store."""
                for f in range(NFG):
                    sl = slice(f * FG, (f + 1) * FG)
                    h_sb = bw.tile([P, KH, FG], f8, tag="h")
                    for mh in range(KH):
                        ph = ps.tile([P, FG], f32, tag="pb")
                        nc.tensor.matmul(
                            ph[:], w1_sb[:, 0:2, mh * P:(mh + 1) * P],
                            yn[:, 0:2, sl], start=True, stop=False,
                            perf_mode=mybir.MatmulPerfMode.DoubleRow)
                        nc.tensor.matmul(
                            ph[:], w1_sb[:, 2, mh * P:(mh + 1) * P],
                            yn[:, 2, sl], start=False, stop=True)
                        nc.scalar.activation(
                            h_sb[:, mh, :], ph[:], AF.Gelu, scale=1.0 / f8sc)
                    o_sb = bw.tile([P, KS, FG], bf16, tag="o", bufs=1)
                    for mo in range(KS):
                        po = ps.tile([P, FG], f32, tag="pb")
                        for sp in range(KH // 2):
                            nc.tensor.matmul(
                                po[:], w2_sb[:, 2 * sp:2 * sp + 2,
                                             mo * P:(mo + 1) * P],
                                h_sb[:, 2 * sp:2 * sp + 2, :],
                                start=(sp == 0), stop=(sp == KH // 2 - 1),
                                perf_mode=mybir.MatmulPerfMode.DoubleRow)
                        ff = bw.tile([P, FG], bf16, tag="ff", bufs=1)
                        nc.vector.tensor_scalar_mul(ff[:], po[:], 1.0 / f8sc)
                        nc.vector.tensor_add(o_sb[:, mo, :], ff[:], y[:, mo, sl])
                    nc.sync.dma_start(out_r[img][:, :, sl], o_sb[:])

            # ----------------- schedule -----------------
            acc0 = alloc_acc()
            phase_a(0, acc0)
            stages0, st0 = attn_stages(0, *acc0)
            accB = alloc_acc()
            phase_a(1, accB, interleave=stages0)
            stages1, st1 = attn_stages(1, *accB)
            y0, yn0 = phase_b1(0, st0, interleave=stages1)
            phase_b2(0, y0, yn0)
            y1, yn1 = phase_b1(1, st1)
            phase_b2(1, y1, yn1)

    return _split_waits(nc)


def _prep_weights(inputs):
    import ml_dtypes
    bf = ml_dtypes.bfloat16
    f8 = ml_dtypes.float8_e4m3
    w_qkv = np.asarray(inputs["w_qkv"], np.float32)
    g1 = np.asarray(inputs["g1"], np.float32)
    g2 = np.asarray(inputs["g2"], np.float32)
    for name in ("beta1", "beta2", "b_qkv", "b_proj", "b_ffn1", "b_ffn2"):
        assert not np.any(np.asarray(inputs[name])), f"{name} nonzero unsupported"
    wg = w_qkv * g1[None, :]  # fold LN gamma into qkv weights
    wg3 = wg.reshape(NH, 3 * CH, C)
    wq = wg3[:, 0:CH, :]
    wk = wg3[:, CH:2 * CH, :]
    wv_ = wg3[:, 2 * CH:3 * CH, :]
    # qk columns interleaved per head: j = h*96 + (0..47 q | 48..95 k)
    wqk = np.concatenate([wq, wk], axis=1).reshape(2 * C, C)
    wqk_t = np.ascontiguousarray(wqk.T)  # [384, 768]
    u_qk = wqk.sum(axis=1)[None, :]  # [1, 768]
    wv_t = np.ascontiguousarray(wv_.transpose(1, 0, 2))  # [48, NH, 384]
    wpj48 = np.ascontiguousarray(
        np.asarray(inputs["w_proj"], np.float32).T.reshape(NH, CH, C)
        .transpose(1, 0, 2))
    w1g = np.asarray(inputs["w_ffn1"], np.float32) * g2[None, :]
    w1_t = np.ascontiguousarray(w1g.T)  # [384, 1536]
    w2_t = np.ascontiguousarray(np.asarray(inputs["w_ffn2"], np.float32).T)
    ls = np.asarray(inputs["logit_scale"], np.float32).reshape(NH)
    scale_row = np.exp(np.minimum(ls, LOGIT_MAX))[None, :]
    # ffn weights scaled by 64 into fp8 e4m3 (compensated at eviction) to
    # stay clear of the e4m3 denormal floor (2^-6)
    return dict(
        wqk_t=wqk_t.astype(bf), u_qk=np.ascontiguousarray(u_qk).astype(bf),
        wv=wv_t.astype(bf), wpj48=wpj48.astype(bf),
        w1_t=(w1_t * 64.0).astype(f8), w2_t=(w2_t * 64.0).astype(f8),
        scale_row=np.ascontiguousarray(scale_row).astype(np.float32))


def _make_in_maps(inputs):
    import ml_dtypes
    x = np.asarray(inputs["x"], np.float32).reshape(B, C, N).astype(
        ml_dtypes.bfloat16)
    wmap = _prep_weights(inputs)
    in_maps = []
    for c in range(NCORES):
        m = dict(wmap)
        m["xs"] = np.ascontiguousarray(x[c * BPC:(c + 1) * BPC])
        in_maps.append(m)
    return in_maps


def kernel(**inputs):
    from concourse.bass_utils import run_bass_kernel_spmd

    if "nc" not in _CACHE:
        _CACHE["nc"] = _build_nc()
    nc = _CACHE["nc"]
    in_maps = _make_in_maps(inputs)
    res = run_bass_kernel_spmd(nc, in_maps, list(range(NCORES)))
    out = np.concatenate(
        [np.asarray(r["out"], np.float32) for r in res.results], axis=0)
    return out.reshape(B, C, 64, 64)


# revision 20
# speedup vs baseline: 1.9939x; 1.2012x over previous
"""Trainium2 Bass kernel for nn_CATransformer1 (XCiT-style channel-attention block).

Sharding: data-parallel over batch. 16 images / 8 cores = 2 images per core.
Weights replicated; no collectives.

V2 design (bf16 everywhere):
  - x is DMAed once per image (bf16) and stays SBUF-resident across both
    passes; output written back as bf16 and widened on host.
  - All matmuls run in bf16 (full rate at any free-dim size on TRN2).
  - LN1/LN2 stats are computed in column form (per-pixel partitions) with
    F=1 ones-matmuls (nearly free on the PE), then transposed to row form
    for the rank-1 mean terms and row-broadcasts.
  - LN1 mean is folded into the QKV matmul as a rank-1 K=1 accumulation
    (lhsT=mneg row, rhs=u row); rstd applied at PSUM eviction via
    per-partition tensor_scalar (pixels are partitions).
  - Attention output + projection collapsed into per-image G = Wproj @
    concat_h(attn_h @ Wv_h); attn branch = rstd * (G@x - m*uG) via the same
    rank-1 trick; LN2 materializes yn (bf16) for the FFN.
  - Eviction work split across DVE and Activation engines; emission is
    software-pipelined (S-accum deferred one chunk; image-1 attention block
    interleaved into image-0 phase B) so the PE stays fed.
"""

import numpy as np

B, C, NH, CH, N, HID = 16, 384, 8, 48, 4096, 1536
NCORES = 8
BPC = B // NCORES  # images per core
P = 128
KS = C // P   # 3 k-subtiles for C
KH = HID // P  # 12 k-subtiles for HID
NT = N // P   # 32 pixel chunks (phase A)
FG = 512      # phase B pixel chunk
NFG = N // FG
LOGIT_MAX = float(np.log(1.0 / 0.01))
EPS_LN = 1e-5
EPS_NORM = 1e-12

_CACHE = {}


def _patch_tile_drain():
    """Walrus in this env rejects >1 sync-wait on the kernel-tail Drain
    (CTRL_NO_STRUCT setupSyncWait).  Split the waits across a chain of
    drain instructions, one wait each.  Idempotent, in-process only."""
    import concourse.tile as tile
    from concourse import mybir
    from concourse.vector_clock import ScopedClock

    if getattr(tile.TileContext._drain_and_barrier, "_split_patch", False):
        return

    def _split_drain(self, tick_clock, wait_clock):
        drain_inst = self.nc.sync.drain()
        wait_clock.add_sem_waits(
            drain_inst.ins, ScopedClock({None: tick_clock.global_clock}))
        si = drain_inst.ins.sync_info
        if si is not None and si.on_wait and len(si.on_wait) > 1:
            waits = list(si.on_wait)
            si.on_wait = waits[:1]
            for w in waits[1:]:
                d2 = self.nc.sync.drain()
                d2.ins.sync_info = mybir.SyncInfo(on_wait=[w], on_update=[])
        self.nc.all_engine_barrier()
        popped = self.nc._tile_sem_poison_stack.pop()
        assert popped is self._sem_poison
        self.nc.clear_and_free_semaphores(list(self.sems.allocated().values()))
        self.nc.all_engine_barrier()

    _split_drain._split_patch = True
    tile.TileContext._drain_and_barrier = _split_drain


def _split_waits(nc, max_waits=1):
    """This walrus build rejects instructions carrying more than one sync
    wait ('Too many sync wait commands' / 'ISA wrong length').  Move extra
    waits onto same-engine NoOps inserted immediately before."""
    from concourse import mybir

    n = 0
    for fn in nc.m.functions:
        for blk in fn.blocks:
            out = []
            for inst in blk.instructions:
                si = inst.sync_info
                # custom-DVE InstISA can't carry any sync commands at all
                mw = 0 if isinstance(inst, mybir.InstISA) else max_waits
                if si is not None and si.on_wait and len(si.on_wait) > mw:
                    waits = list(si.on_wait)
                    keep = waits[-mw:] if mw else []
                    for w in waits[:len(waits) - mw]:
                        n += 1
                        nop = mybir.InstNoOp(
                            name=f"I-wsplit-{n}", ins=[], outs=[])
                        nop.engine = inst.engine
                        nop.sync_info = mybir.SyncInfo(
                            on_wait=[w], on_update=[])
                        out.append(nop)
                    si.on_wait = keep
                out.append(inst)
                if (isinstance(inst, mybir.InstISA) and si is not None
                        and si.on_update):
                    n += 1
                    nop = mybir.InstNoOp(name=f"I-usplit-{n}", ins=[], outs=[])
                    nop.engine = inst.engine
                    nop.sync_info = mybir.SyncInfo(
                        on_wait=[], on_update=list(si.on_update))
                    out.append(nop)
                    si.on_update = []
            blk.instructions = out
    return nc


def _build_nc():
    import concourse.bass as bass
    import concourse.tile as tile
    from concourse import mybir

    dt = mybir.dt
    AF = mybir.ActivationFunctionType
    ALU = mybir.AluOpType
    AX = mybir.AxisListType
    from concourse.masks import make_identity

    f32 = dt.float32
    bf16 = dt.bfloat16

    _patch_tile_drain()
    nc = bass.Bass()

    xs = nc.declare_dram_parameter("xs", [BPC, C, N], bf16, isOutput=False)
    wqk_t = nc.declare_dram_parameter("wqk_t", [C, 2 * C], bf16, isOutput=False)
    u_qk = nc.declare_dram_parameter("u_qk", [1, 2 * C], bf16, isOutput=False)
    wv = nc.declare_dram_parameter("wv", [CH, NH, C], bf16, isOutput=False)
    wpj48 = nc.declare_dram_parameter("wpj48", [CH, NH, C], bf16, isOutput=False)
    f8 = dt.float8e4
    w1_t = nc.declare_dram_parameter("w1_t", [C, HID], f8, isOutput=False)
    w2_t = nc.declare_dram_parameter("w2_t", [HID, C], f8, isOutput=False)
    scale_row = nc.declare_dram_parameter("scale_row", [1, NH], f32, isOutput=False)
    out_d = nc.declare_dram_parameter("out", [BPC, C, N], bf16, isOutput=True)

    with tile.TileContext(nc) as tc:
        with (
            tc.tile_pool(name="consts", bufs=1) as consts,
            tc.tile_pool(name="ximg", bufs=2) as xpool,
            tc.tile_pool(name="qkp", bufs=2) as qkpool,
            tc.tile_pool(name="attn", bufs=2) as apool,
            tc.tile_pool(name="scr", bufs=3) as scr,
            tc.tile_pool(name="bwork", bufs=2) as bw,
            tc.tile_pool(name="pb", bufs=6, space="PSUM") as ps,
            tc.tile_pool(name="acc", bufs=1, space="PSUM") as psacc,
        ):
            def bcast_read(dst, dram_row, parts):
                src = bass.AP(
                    tensor=dram_row.tensor, offset=dram_row.offset,
                    ap=[[0, parts]] + [list(d) for d in dram_row.ap[-1:]])
                nc.gpsimd.dma_start(dst, src)

            # ---------------- constants ----------------
            wqk_sb = consts.tile([P, KS, 2 * C], bf16, tag="wqk")
            nc.gpsimd.dma_start(wqk_sb[:], wqk_t.rearrange("(s p) f -> p s f", p=P))
            wv_sb = consts.tile([CH, NH, C], bf16, tag="wv")
            nc.gpsimd.dma_start(wv_sb[:], wv[:])
            wpj_sb = consts.tile([CH, NH, C], bf16, tag="wpj")
            nc.gpsimd.dma_start(wpj_sb[:], wpj48[:])
            w1_sb = consts.tile([P, KS, HID], f8, tag="w1")
            nc.gpsimd.dma_start(w1_sb[:], w1_t.rearrange("(s p) f -> p s f", p=P))
            w2_sb = consts.tile([P, KH, C], f8, tag="w2")
            nc.gpsimd.dma_start(w2_sb[:], w2_t.rearrange("(s p) f -> p s f", p=P))
            uqk_sb = consts.tile([1, 2 * C], bf16, tag="uqk")
            nc.gpsimd.dma_start(uqk_sb[:], u_qk[:])
            ones_col = consts.tile([P, 1], bf16, tag="onescol")
            nc.vector.memset(ones_col[:], 1.0)
            ones_row = consts.tile([1, P], bf16, tag="onesrow")
            nc.vector.memset(ones_row[:], 1.0)
            identb = consts.tile([P, P], bf16, tag="identb")
            make_identity(nc, identb[:])
            schb = consts.tile([CH, NH], f32, tag="schb")
            bcast_read(schb[:], scale_row[0, :], parts=CH)

            xs_r = xs.rearrange("b (s p) n -> b p s n", p=P)
            out_r = out_d.rearrange("b (s p) n -> b p s n", p=P)

            # ------------- load both images upfront -------------
            x_tiles, rowpairs = [], []
            for img in range(BPC):
                x_sb = xpool.tile([P, KS, N], bf16, tag="x")
                for i in range(8):
                    sl = slice(i * 512, (i + 1) * 512)
                    nc.sync.dma_start(x_sb[:, :, sl], xs_r[img][:, :, sl])
                x_tiles.append(x_sb)
                # LN1 per-pixel rows: -mean and rstd (partition 0)
                mrow = xpool.tile([1, N], bf16, tag="mrow")
                rrow = xpool.tile([1, N], bf16, tag="rrow")
                rowpairs.append((mrow, rrow))

            def alloc_acc():
                acc1 = psacc.tile([CH, 400], f32, tag="acc1")  # S | q-norms²
                acc2 = psacc.tile([1, C], f32, tag="acc2")     # k-norms² row
                return acc1, acc2

            def phase_a(img, acc, interleave=()):
                """LN1 stats + qkT + S/norm accumulation for one image.
                Stats run one chunk ahead of qkT so the PE never waits on
                the stats DVE chain; S-accum is deferred one chunk behind."""
                x_sb = x_tiles[img]
                mrow, rrow = rowpairs[img]
                acc1, acc2 = acc

                def stats_mm(t):
                    sl = slice(t * P, (t + 1) * P)
                    xsq = scr.tile([P, KS, P], bf16, tag="xsq", bufs=2)
                    nc.vector.tensor_mul(xsq[:], x_sb[:, :, sl], x_sb[:, :, sl])
                    pstat = ps.tile([P, 2], f32, tag="pb")
                    for s in range(KS):
                        nc.tensor.matmul(
                            pstat[:, 0:1], x_sb[:, s, sl], ones_col[:],
                            start=(s == 0), stop=(s == KS - 1))
                    for s in range(KS):
                        nc.tensor.matmul(
                            pstat[:, 1:2], xsq[:, s, :], ones_col[:],
                            start=(s == 0), stop=(s == KS - 1))
                    stat2 = scr.tile([P, 33], bf16, tag="stat2")
                    vcol = scr.tile([P, 1], f32, tag="vcol")
                    msq = scr.tile([P, 1], f32, tag="msq")
                    rcol = scr.tile([P, 1], f32, tag="rcol")
                    nc.scalar.activation(
                        stat2[:, 0:1], pstat[:, 0:1], AF.Copy, scale=-1.0 / C)
                    nc.vector.tensor_scalar(
                        vcol[:], pstat[:, 1:2], 1.0 / C, EPS_LN,
                        op0=ALU.mult, op1=ALU.add)
                    nc.scalar.activation(msq[:], stat2[:, 0:1], AF.Square)
                    nc.vector.tensor_sub(vcol[:], vcol[:], msq[:])
                    nc.scalar.activation(rcol[:], vcol[:], AF.Sqrt)
                    nc.vector.reciprocal(rcol[:], rcol[:])
                    nc.vector.tensor_copy(stat2[:, 32:33], rcol[:])
                    return stat2, rcol

                def stats_tr(t, stat2):
                    sl = slice(t * P, (t + 1) * P)
                    ptr = ps.tile([33, P], bf16, tag="pb")
                    nc.tensor.transpose(ptr[:], stat2[:], identb[:])
                    nc.scalar.copy(mrow[0:1, sl], ptr[0:1, :])
                    nc.scalar.copy(rrow[0:1, sl], ptr[32:33, :])

                pend = None
                nxt = stats_mm(0)
                stats_tr(0, nxt[0])
                for t in range(NT):
                    if 1 <= t <= len(interleave):
                        interleave[t - 1]()
                    sl = slice(t * P, (t + 1) * P)
                    rcol = nxt[1]
                    if t + 1 < NT:
                        nxt = stats_mm(t + 1)
                    # qkT x-part into PSUM (two banks)
                    pa1 = ps.tile([P, 512], f32, tag="pb")
                    pa2 = ps.tile([P, 256], f32, tag="pb")
                    for s in range(KS):
                        nc.tensor.matmul(
                            pa1[:], x_sb[:, s, sl], wqk_sb[:, s, 0:512],
                            start=(s == 0), stop=False)
                    for s in range(KS):
                        nc.tensor.matmul(
                            pa2[:], x_sb[:, s, sl], wqk_sb[:, s, 512:768],
                            start=(s == 0), stop=False)
                    if t + 1 < NT:
                        stats_tr(t + 1, nxt[0])
                    # rank-1 mean completion (rows for chunk t are ready)
                    nc.tensor.matmul(
                        pa1[:], mrow[0:1, sl], uqk_sb[0:1, 0:512],
                        start=False, stop=True)
                    nc.tensor.matmul(
                        pa2[:], mrow[0:1, sl], uqk_sb[0:1, 512:768],
                        start=False, stop=True)
                    # deferred S/norm accumulation from previous chunk
                    if pend is not None:
                        _emit_s(acc1, acc2, *pend)
                    # evictions: qk = rstd*pa (DVE + ACT), qksq = qk² (DVE)
                    qk = qkpool.tile([P, 2 * C], bf16, tag="qk")
                    qksq = qkpool.tile([P, 2 * C], bf16, tag="qksq")
                    nc.vector.tensor_scalar_mul(qk[:, 0:512], pa1[:], rcol[:])
                    nc.scalar.activation(
                        qk[:, 512:768], pa2[:], AF.Copy, scale=rcol[:])
                    nc.vector.tensor_mul(qksq[:], qk[:], qk[:])
                    pend = (qk, qksq, t)
                _emit_s(acc1, acc2, *pend)

            def _emit_s(acc1, acc2, qk, qksq, t):
                st, sp = (t == 0), (t == NT - 1)
                for h in range(NH):
                    o = h * 2 * CH
                    nc.tensor.matmul(
                        acc1[:, h * CH:(h + 1) * CH],
                        qk[:, o:o + CH], qk[:, o + CH:o + 2 * CH],
                        start=st, stop=sp)
                for h in range(NH):
                    o = h * 2 * CH
                    nc.tensor.matmul(
                        acc1[:, 384 + h:385 + h],
                        qksq[:, o:o + CH], ones_col[:],
                        start=st, stop=sp)
                ksq = qksq.rearrange("p (h two c) -> p h two c", two=2, c=CH)
                nc.tensor.matmul(
                    acc2[:], ones_col[:], ksq[:, :, 1, :], start=st, stop=sp)

            def attn_stages(img, acc1, acc2):
                """Softmax + G build as a list of emission closures."""
                st = {}

                def s0():  # norms + scaled S + softmax -> sSb (bf16)
                    rq = apool.tile([CH, NH], f32, tag="rq", bufs=1)
                    nc.scalar.activation(rq[:], acc1[:, 384:392], AF.Sqrt)
                    nc.vector.tensor_scalar_max(rq[:], rq[:], EPS_NORM)
                    nc.vector.reciprocal(rq[:], rq[:])
                    nc.vector.tensor_mul(rq[:], rq[:], schb[:])
                    rk = apool.tile([1, C], f32, tag="rk", bufs=1)
                    nc.scalar.activation(rk[:], acc2[:], AF.Sqrt)
                    nc.vector.tensor_scalar_max(rk[:], rk[:], EPS_NORM)
                    nc.vector.reciprocal(rk[:], rk[:])
                    rkb = apool.tile([1, C], bf16, tag="rkb", bufs=1)
                    nc.vector.tensor_copy(rkb[:], rk[:])
                    rkb_ps = ps.tile([CH, C], f32, tag="pb")
                    nc.tensor.matmul(
                        rkb_ps[:], ones_row[0:1, 0:CH], rkb[0:1, :],
                        start=True, stop=True)
                    sS = apool.tile([CH, NH, CH], f32, tag="sS", bufs=1)
                    s_v = acc1[:, 0:384].rearrange("p (h e) -> p h e", e=CH)
                    nc.vector.tensor_mul(
                        sS[:], s_v, rq[:, :, None].to_broadcast((CH, NH, CH)))
                    rkb_v = rkb_ps.rearrange("p (h e) -> p h e", e=CH)
                    nc.vector.tensor_mul(sS[:], sS[:], rkb_v)
                    mx = apool.tile([CH, NH], f32, tag="mx", bufs=1)
                    nc.vector.reduce_max(mx[:], sS[:], axis=AX.X)
                    nc.vector.tensor_sub(
                        sS[:], sS[:], mx[:, :, None].to_broadcast((CH, NH, CH)))
                    nc.scalar.activation(sS[:], sS[:], AF.Exp)
                    esum = apool.tile([CH, NH], f32, tag="esum", bufs=1)
                    nc.vector.reduce_sum(esum[:], sS[:], axis=AX.X)
                    nc.vector.reciprocal(esum[:], esum[:])
                    sSb = apool.tile([CH, NH, CH], bf16, tag="sSb", bufs=1)
                    nc.vector.tensor_mul(
                        sSb[:], sS[:],
                        esum[:, :, None].to_broadcast((CH, NH, CH)))
                    st["sSb"] = sSb

                def s1():  # transpose attn per head
                    pt8 = ps.tile([CH, NH, CH], bf16, tag="pb")
                    for h in range(NH):
                        nc.tensor.transpose(
                            pt8[:, h, :], st["sSb"][:, h, :], identb[0:CH, 0:CH])
                    atT = apool.tile([CH, NH, CH], bf16, tag="atT", bufs=1)
                    nc.vector.tensor_copy(atT[:], pt8[:])
                    st["atT"] = atT

                def s2():  # awv_h = attn_h @ Wv_h
                    awv = apool.tile([CH, NH, C], bf16, tag="awv", bufs=1)
                    for h in range(NH):
                        paw = ps.tile([CH, C], f32, tag="pb")
                        nc.tensor.matmul(
                            paw[:], st["atT"][:, h, :], wv_sb[:, h, :],
                            start=True, stop=True)
                        if h % 2 == 0:
                            nc.vector.tensor_copy(awv[:, h, :], paw[:])
                        else:
                            nc.scalar.copy(awv[:, h, :], paw[:])
                    st["awv"] = awv

                def s3():  # G^T
                    gt_sb = apool.tile([P, KS, C], bf16, tag="gt")
                    for j in range(KS):
                        pgt = ps.tile([P, C], f32, tag="pb")
                        for h in range(NH):
                            nc.tensor.matmul(
                                pgt[:], st["awv"][:, h, j * P:(j + 1) * P],
                                wpj_sb[:, h, :], start=(h == 0), stop=(h == NH - 1))
                        if j % 2 == 0:
                            nc.vector.tensor_copy(gt_sb[:, j, :], pgt[:])
                        else:
                            nc.scalar.copy(gt_sb[:, j, :], pgt[:])
                    st["gt"] = gt_sb

                def s4():  # uG row
                    pug = ps.tile([1, C], f32, tag="pb")
                    for s in range(KS):
                        nc.tensor.matmul(
                            pug[:], ones_col[:], st["gt"][:, s, :],
                            start=(s == 0), stop=(s == KS - 1))
                    ug = apool.tile([1, C], bf16, tag="ug")
                    nc.vector.tensor_copy(ug[:], pug[:])
                    st["ug"] = ug

                return [s0, s1, s2, s3, s4], st

            # per-image y / yn tiles (yn in fp8 for the DR ffn)
            f8sc = 64.0  # host scales w1/w2 by 64 (fp8 e4m3 denormal floor)

            def phase_b1(img, st, interleave=()):
                """G-branch apply + residual + LN2; produces y (bf16) and
                yn (fp8) for the whole image."""
                mrow, rrow = rowpairs[img]
                # y reuses the x image slots (x residency ends with phase A;
                # B1 re-reads x chunk-wise from DRAM)
                y = xpool.tile([P, KS, N], bf16, tag="x", name=f"y{img}")
                yn = bw.tile([P, KS, N], f8, tag="yn", bufs=1)
                gt, ug = st["gt"], st["ug"]
                pends, pend2 = [], []
                for f in range(NFG):
                    if f < len(interleave):
                        interleave[f]()
                    sl = slice(f * FG, (f + 1) * FG)
                    xb = scr.tile([P, KS, FG], bf16, tag="xb", bufs=2)
                    nc.sync.dma_start(xb[:], xs_r[img][:, :, sl])
                    bc1 = ps.tile([P, FG], f32, tag="pb")
                    nc.tensor.matmul(
                        bc1[:], ones_row[0:1, :], rrow[0:1, sl],
                        start=True, stop=True)
                    rb = scr.tile([P, FG], bf16, tag="rb", bufs=2)
                    nc.scalar.copy(rb[:], bc1[:])
                    pgs = []
                    for j in range(KS):
                        pg = ps.tile([P, FG], f32, tag="pb", name=f"pg{j}")
                        for s in range(KS):
                            nc.tensor.matmul(
                                pg[:], gt[:, s, j * P:(j + 1) * P],
                                xb[:, s, :], start=(s == 0), stop=False)
                        nc.tensor.matmul(
                            pg[:], ug[0:1, j * P:(j + 1) * P], mrow[0:1, sl],
                            start=False, stop=True)
                        pgs.append(pg)
                    ab = bw.tile([P, KS, FG], bf16, tag="ab", bufs=1)
                    for j in range(KS):
                        nc.vector.tensor_mul(ab[:, j, :], pgs[j][:], rb[:])
                    nc.vector.tensor_add(y[:, :, sl], xb[:], ab[:])
                    ysq = bw.tile([P, KS, FG], bf16, tag="ysq", bufs=2)
                    nc.vector.tensor_mul(ysq[:], y[:, :, sl], y[:, :, sl])
                    # 2-deep pipeline: stats one chunk behind, apply two
                    if f >= 1:
                        pend2.append(_ln2_stats(img, y, f - 1, pends[f - 1]))
                    if f >= 2:
                        _ln2_apply(img, y, yn, f - 2, pend2[f - 2])
                    pends.append(ysq)
                pend2.append(_ln2_stats(img, y, NFG - 1, pends[NFG - 1]))
                _ln2_apply(img, y, yn, NFG - 2, pend2[NFG - 2])
                _ln2_apply(img, y, yn, NFG - 1, pend2[NFG - 1])
                return y, yn

            def _ln2_stats(img, y, f, ysq):
                sl = slice(f * FG, (f + 1) * FG)
                p2a = ps.tile([1, FG], f32, tag="pb")
                p2b = ps.tile([1, FG], f32, tag="pb")
                for s in range(KS):
                    nc.tensor.matmul(
                        p2a[:], ones_col[:], y[:, s, sl],
                        start=(s == 0), stop=(s == KS - 1))
                for s in range(KS):
                    nc.tensor.matmul(
                        p2b[:], ones_col[:], ysq[:, s, :],
                        start=(s == 0), stop=(s == KS - 1))
                m2b = scr.tile([1, FG], bf16, tag="m2b", bufs=2)
                nc.scalar.activation(m2b[:], p2a[:], AF.Copy, scale=-1.0 / C)
                vrow = scr.tile([1, FG], f32, tag="vrow", bufs=2)
                nc.vector.tensor_scalar(
                    vrow[:], p2b[:], 1.0 / C, EPS_LN, op0=ALU.mult, op1=ALU.add)
                msq = scr.tile([1, FG], f32, tag="msqr", bufs=2)
                nc.scalar.activation(msq[:], m2b[:], AF.Square)
                nc.vector.tensor_sub(vrow[:], vrow[:], msq[:])
                srow = scr.tile([1, FG], f32, tag="srow", bufs=2)
                nc.scalar.activation(srow[:], vrow[:], AF.Sqrt)
                r2f = scr.tile([1, FG], f32, tag="r2f", bufs=2)
                nc.vector.reciprocal(r2f[:], srow[:])
                r2b = scr.tile([1, FG], bf16, tag="r2b", bufs=2)
                nc.vector.tensor_copy(r2b[:], r2f[:])
                return m2b, r2b

            def _ln2_apply(img, y, yn, f, rows):
                sl = slice(f * FG, (f + 1) * FG)
                m2b, r2b = rows
                bcm = ps.tile([P, FG], f32, tag="pb")
                nc.tensor.matmul(
                    bcm[:], ones_row[0:1, :], m2b[0:1, :], start=True, stop=True)
                bcr = ps.tile([P, FG], f32, tag="pb")
                nc.tensor.matmul(
                    bcr[:], ones_row[0:1, :], r2b[0:1, :], start=True, stop=True)
                mbc = scr.tile([P, FG], bf16, tag="mbc", bufs=2)
                nc.scalar.copy(mbc[:], bcm[:])
                rbc = scr.tile([P, FG], bf16, tag="rbc", bufs=2)
                nc.scalar.copy(rbc[:], bcr[:])
                t3 = bw.tile([P, KS, FG], bf16, tag="t3", bufs=1)
                nc.vector.tensor_add(
                    t3[:], y[:, :, sl], mbc[:, None, :].to_broadcast((P, KS, FG)))
                nc.vector.tensor_mul(
                    yn[:, :, sl], t3[:], rbc[:, None, :].to_broadcast((P, KS, FG)))

            def phase_b2(img, y, yn):
                """FFN in fp8 DoubleRow + residual + store."""
                for f in range(NFG):
                    sl = slice(f * FG, (f + 1) * FG)
                    h_sb = bw.tile([P, KH, FG], f8, tag="h")
                    for mh in range(KH):
                        ph = ps.tile([P, FG], f32, tag="pb")
                        nc.tensor.matmul(
                            ph[:], w1_sb[:, 0:2, mh * P:(mh + 1) * P],
                            yn[:, 0:2, sl], start=True, stop=False,
                            perf_mode=mybir.MatmulPerfMode.DoubleRow)
                        nc.tensor.matmul(
                            ph[:], w1_sb[:, 2, mh * P:(mh + 1) * P],
                            yn[:, 2, sl], start=False, stop=True)
                        nc.scalar.activation(
                            h_sb[:, mh, :], ph[:], AF.Gelu, scale=1.0 / f8sc)
                    o_sb = bw.tile([P, KS, FG], bf16, tag="o", bufs=1)
                    for mo in range(KS):
                        po = ps.tile([P, FG], f32, tag="pb")
                        for sp in range(KH // 2):
                            nc.tensor.matmul(
                                po[:], w2_sb[:, 2 * sp:2 * sp + 2,
                                             mo * P:(mo + 1) * P],
                                h_sb[:, 2 * sp:2 * sp + 2, :],
                                start=(sp == 0), stop=(sp == KH // 2 - 1),
                                perf_mode=mybir.MatmulPerfMode.DoubleRow)
                        ff = bw.tile([P, FG], bf16, tag="ff", bufs=1)
                        nc.vector.tensor_scalar_mul(ff[:], po[:], 1.0 / f8sc)
                        nc.vector.tensor_add(o_sb[:, mo, :], ff[:], y[:, mo, sl])
                    nc.sync.dma_start(out_r[img][:, :, sl], o_sb[:])

            # ----------------- schedule -----------------
            acc0 = alloc_acc()
            phase_a(0, acc0)
            stages0, st0 = attn_stages(0, *acc0)
            accB = alloc_acc()
            phase_a(1, accB, interleave=stages0)
            stages1, st1 = attn_stages(1, *accB)
            y0, yn0 = phase_b1(0, st0, interleave=stages1)
            phase_b2(0, y0, yn0)
            y1, yn1 = phase_b1(1, st1)
            phase_b2(1, y1, yn1)

    return _split_waits(nc)


def _prep_weights(inputs):
    import ml_dtypes
    bf = ml_dtypes.bfloat16
    f8 = ml_dtypes.float8_e4m3
    w_qkv = np.asarray(inputs["w_qkv"], np.float32)
    g1 = np.asarray(inputs["g1"], np.float32)
    g2 = np.asarray(inputs["g2"], np.float32)
    for name in ("beta1", "beta2", "b_qkv", "b_proj", "b_ffn1", "b_ffn2"):
        assert not np.any(np.asarray(inputs[name])), f"{name} nonzero unsupported"
    wg = w_qkv * g1[None, :]  # fold LN gamma into qkv weights
    wg3 = wg.reshape(NH, 3 * CH, C)
    wq = wg3[:, 0:CH, :]
    wk = wg3[:, CH:2 * CH, :]
    wv_ = wg3[:, 2 * CH:3 * CH, :]
    # qk columns interleaved per head: j = h*96 + (0..47 q | 48..95 k)
    wqk = np.concatenate([wq, wk], axis=1).reshape(2 * C, C)
    wqk_t = np.ascontiguousarray(wqk.T)  # [384, 768]
    u_qk = wqk.sum(axis=1)[None, :]  # [1, 768]
    wv_t = np.ascontiguousarray(wv_.transpose(1, 0, 2))  # [48, NH, 384]
    wpj48 = np.ascontiguousarray(
        np.asarray(inputs["w_proj"], np.float32).T.reshape(NH, CH, C)
        .transpose(1, 0, 2))
    w1g = np.asarray(inputs["w_ffn1"], np.float32) * g2[None, :]
    w1_t = np.ascontiguousarray(w1g.T)  # [384, 1536]
    w2_t = np.ascontiguousarray(np.asarray(inputs["w_ffn2"], np.float32).T)
    ls = np.asarray(inputs["logit_scale"], np.float32).reshape(NH)
    scale_row = np.exp(np.minimum(ls, LOGIT_MAX))[None, :]
    # ffn weights scaled by 64 into fp8 e4m3 (compensated at eviction) to
    # stay clear of the e4m3 denormal floor (2^-6)
    return dict(
        wqk_t=wqk_t.astype(bf), u_qk=np.ascontiguousarray(u_qk).astype(bf),
        wv=wv_t.astype(bf), wpj48=wpj48.astype(bf),
        w1_t=(w1_t * 64.0).astype(f8), w2_t=(w2_t * 64.0).astype(f8),
        scale_row=np.ascontiguousarray(scale_row).astype(np.float32))


def _make_in_maps(inputs):
    import ml_dtypes
    x = np.asarray(inputs["x"], np.float32).reshape(B, C, N).astype(
        ml_dtypes.bfloat16)
    wmap = _prep_weights(inputs)
    in_maps = []
    for c in range(NCORES):
        m = dict(wmap)
        m["xs"] = np.ascontiguousarray(x[c * BPC:(c + 1) * BPC])
        in_maps.append(m)
    return in_maps


def kernel(**inputs):
    from concourse.bass_utils import run_bass_kernel_spmd

    if "nc" not in _CACHE:
        _CACHE["nc"] = _build_nc()
    nc = _CACHE["nc"]
    in_maps = _make_in_maps(inputs)
    res = run_bass_kernel_spmd(nc, in_maps, list(range(NCORES)))
    out = np.concatenate(
        [np.asarray(r["out"], np.float32) for r in res.results], axis=0)
    return out.reshape(B, C, 64, 64)


# revision 24
# speedup vs baseline: 2.2375x; 1.1222x over previous
"""Trainium2 Bass kernel for nn_CATransformer1 (XCiT-style channel-attention block).

Sharding: data-parallel over batch. 16 images / 8 cores = 2 images per core.
Weights replicated; no collectives.

V2 design (bf16 everywhere):
  - x is DMAed once per image (bf16) and stays SBUF-resident across both
    passes; output written back as bf16 and widened on host.
  - All matmuls run in bf16 (full rate at any free-dim size on TRN2).
  - LN1/LN2 stats are computed in column form (per-pixel partitions) with
    F=1 ones-matmuls (nearly free on the PE), then transposed to row form
    for the rank-1 mean terms and row-broadcasts.
  - LN1 mean is folded into the QKV matmul as a rank-1 K=1 accumulation
    (lhsT=mneg row, rhs=u row); rstd applied at PSUM eviction via
    per-partition tensor_scalar (pixels are partitions).
  - Attention output + projection collapsed into per-image G = Wproj @
    concat_h(attn_h @ Wv_h); attn branch = rstd * (G@x - m*uG) via the same
    rank-1 trick; LN2 materializes yn (bf16) for the FFN.
  - Eviction work split across DVE and Activation engines; emission is
    software-pipelined (S-accum deferred one chunk; image-1 attention block
    interleaved into image-0 phase B) so the PE stays fed.
"""

import numpy as np

B, C, NH, CH, N, HID = 16, 384, 8, 48, 4096, 1536
NCORES = 8
BPC = B // NCORES  # images per core
P = 128
KS = C // P   # 3 k-subtiles for C
KH = HID // P  # 12 k-subtiles for HID
NT = N // P   # 32 pixel chunks (phase A)
FG = 512      # phase B pixel chunk
NFG = N // FG
LOGIT_MAX = float(np.log(1.0 / 0.01))
EPS_LN = 1e-5
EPS_NORM = 1e-12

_CACHE = {}


def _patch_tile_drain():
    """Walrus in this env rejects >1 sync-wait on the kernel-tail Drain
    (CTRL_NO_STRUCT setupSyncWait).  Split the waits across a chain of
    drain instructions, one wait each.  Idempotent, in-process only."""
    import concourse.tile as tile
    from concourse import mybir
    from concourse.vector_clock import ScopedClock

    if getattr(tile.TileContext._drain_and_barrier, "_split_patch", False):
        return

    def _split_drain(self, tick_clock, wait_clock):
        drain_inst = self.nc.sync.drain()
        wait_clock.add_sem_waits(
            drain_inst.ins, ScopedClock({None: tick_clock.global_clock}))
        si = drain_inst.ins.sync_info
        if si is not None and si.on_wait and len(si.on_wait) > 1:
            waits = list(si.on_wait)
            si.on_wait = waits[:1]
            for w in waits[1:]:
                d2 = self.nc.sync.drain()
                d2.ins.sync_info = mybir.SyncInfo(on_wait=[w], on_update=[])
        self.nc.all_engine_barrier()
        popped = self.nc._tile_sem_poison_stack.pop()
        assert popped is self._sem_poison
        self.nc.clear_and_free_semaphores(list(self.sems.allocated().values()))
        self.nc.all_engine_barrier()

    _split_drain._split_patch = True
    tile.TileContext._drain_and_barrier = _split_drain


def _split_waits(nc, max_waits=1):
    """This walrus build rejects instructions carrying more than one sync
    wait ('Too many sync wait commands' / 'ISA wrong length').  Move extra
    waits onto same-engine NoOps inserted immediately before."""
    from concourse import mybir

    n = 0
    for fn in nc.m.functions:
        for blk in fn.blocks:
            out = []
            for inst in blk.instructions:
                si = inst.sync_info
                # custom-DVE InstISA can't carry any sync commands at all
                mw = 0 if isinstance(inst, mybir.InstISA) else max_waits
                if si is not None and si.on_wait and len(si.on_wait) > mw:
                    waits = list(si.on_wait)
                    keep = waits[-mw:] if mw else []
                    for w in waits[:len(waits) - mw]:
                        n += 1
                        nop = mybir.InstNoOp(
                            name=f"I-wsplit-{n}", ins=[], outs=[])
                        nop.engine = inst.engine
                        nop.sync_info = mybir.SyncInfo(
                            on_wait=[w], on_update=[])
                        out.append(nop)
                    si.on_wait = keep
                out.append(inst)
                if (isinstance(inst, mybir.InstISA) and si is not None
                        and si.on_update):
                    n += 1
                    nop = mybir.InstNoOp(name=f"I-usplit-{n}", ins=[], outs=[])
                    nop.engine = inst.engine
                    nop.sync_info = mybir.SyncInfo(
                        on_wait=[], on_update=list(si.on_update))
                    out.append(nop)
                    si.on_update = []
            blk.instructions = out
    return nc


def _build_nc():
    import concourse.bass as bass
    import concourse.tile as tile
    from concourse import mybir

    dt = mybir.dt
    AF = mybir.ActivationFunctionType
    ALU = mybir.AluOpType
    AX = mybir.AxisListType
    from concourse.masks import make_identity

    f32 = dt.float32
    bf16 = dt.bfloat16

    _patch_tile_drain()
    nc = bass.Bass()

    xs = nc.declare_dram_parameter("xs", [BPC, C, N], bf16, isOutput=False)
    wqk_t = nc.declare_dram_parameter("wqk_t", [C, 2 * C], bf16, isOutput=False)
    u_qk = nc.declare_dram_parameter("u_qk", [1, 2 * C], bf16, isOutput=False)
    wv = nc.declare_dram_parameter("wv", [CH, NH, C], bf16, isOutput=False)
    wpj48 = nc.declare_dram_parameter("wpj48", [CH, NH, C], bf16, isOutput=False)
    f8 = dt.float8e4
    w1_t = nc.declare_dram_parameter("w1_t", [C, HID], f8, isOutput=False)
    w2_t = nc.declare_dram_parameter("w2_t", [HID, C], f8, isOutput=False)
    scale_row = nc.declare_dram_parameter("scale_row", [1, NH], f32, isOutput=False)
    out_d = nc.declare_dram_parameter("out", [BPC, C, N], bf16, isOutput=True)

    with tile.TileContext(nc) as tc:
        with (
            tc.tile_pool(name="consts", bufs=1) as consts,
            tc.tile_pool(name="ximg", bufs=2) as xpool,
            tc.tile_pool(name="qkp", bufs=2) as qkpool,
            tc.tile_pool(name="attn", bufs=2) as apool,
            tc.tile_pool(name="scr", bufs=3) as scr,
            tc.tile_pool(name="bwork", bufs=2) as bw,
            tc.tile_pool(name="pb", bufs=6, space="PSUM") as ps,
            tc.tile_pool(name="acc", bufs=1, space="PSUM") as psacc,
        ):
            def bcast_read(dst, dram_row, parts):
                src = bass.AP(
                    tensor=dram_row.tensor, offset=dram_row.offset,
                    ap=[[0, parts]] + [list(d) for d in dram_row.ap[-1:]])
                nc.gpsimd.dma_start(dst, src)

            # ---------------- constants ----------------
            wqk_sb = consts.tile([P, KS, 2 * C], bf16, tag="wqk")
            nc.gpsimd.dma_start(wqk_sb[:], wqk_t.rearrange("(s p) f -> p s f", p=P))
            wv_sb = consts.tile([CH, NH, C], bf16, tag="wv")
            nc.gpsimd.dma_start(wv_sb[:], wv[:])
            wpj_sb = consts.tile([CH, NH, C], bf16, tag="wpj")
            nc.gpsimd.dma_start(wpj_sb[:], wpj48[:])
            w1_sb = consts.tile([P, KS, HID], f8, tag="w1")
            nc.gpsimd.dma_start(w1_sb[:], w1_t.rearrange("(s p) f -> p s f", p=P))
            w2_sb = consts.tile([P, KH, C], f8, tag="w2")
            nc.gpsimd.dma_start(w2_sb[:], w2_t.rearrange("(s p) f -> p s f", p=P))
            uqk_sb = consts.tile([1, 2 * C], bf16, tag="uqk")
            nc.gpsimd.dma_start(uqk_sb[:], u_qk[:])
            ones_col = consts.tile([P, 1], bf16, tag="onescol")
            nc.vector.memset(ones_col[:], 1.0)
            ones_row = consts.tile([1, P], bf16, tag="onesrow")
            nc.vector.memset(ones_row[:], 1.0)
            identb = consts.tile([P, P], bf16, tag="identb")
            make_identity(nc, identb[:])
            schb = consts.tile([CH, NH], f32, tag="schb")
            bcast_read(schb[:], scale_row[0, :], parts=CH)

            xs_r = xs.rearrange("b (s p) n -> b p s n", p=P)
            out_r = out_d.rearrange("b (s p) n -> b p s n", p=P)

            # ------------- load both images upfront -------------
            x_tiles, rowpairs = [], []
            for img in range(BPC):
                x_sb = xpool.tile([P, KS, N], bf16, tag="x")
                for i in range(8):
                    sl = slice(i * 512, (i + 1) * 512)
                    nc.sync.dma_start(x_sb[:, :, sl], xs_r[img][:, :, sl])
                x_tiles.append(x_sb)
                # LN1 per-pixel rows: -mean and rstd (partition 0)
                mrow = xpool.tile([1, N], bf16, tag="mrow")
                rrow = xpool.tile([1, N], bf16, tag="rrow")
                rowpairs.append((mrow, rrow))

            def alloc_acc():
                acc1 = psacc.tile([CH, 400], f32, tag="acc1")  # S | q-norms²
                acc2 = psacc.tile([1, C], f32, tag="acc2")     # k-norms² row
                return acc1, acc2

            def phase_a(img, acc, interleave=()):
                """LN1 stats + qkT + S/norm accumulation for one image.
                Stats run one chunk ahead of qkT so the PE never waits on
                the stats DVE chain; S-accum is deferred one chunk behind."""
                x_sb = x_tiles[img]
                mrow, rrow = rowpairs[img]
                acc1, acc2 = acc

                def stats_mm2(t):
                    """Batched LN1 stats for chunks t and t+1."""
                    sl = slice(t * P, (t + 2) * P)
                    xsq = scr.tile([P, KS, 2 * P], bf16, tag="xsq", bufs=2)
                    nc.vector.tensor_mul(xsq[:], x_sb[:, :, sl], x_sb[:, :, sl])
                    pstat = ps.tile([P, 2, 2], f32, tag="pb")
                    for cp in range(2):
                        csl = slice((t + cp) * P, (t + cp + 1) * P)
                        for s in range(KS):
                            nc.tensor.matmul(
                                pstat[:, cp, 0:1], x_sb[:, s, csl], ones_col[:],
                                start=(s == 0), stop=(s == KS - 1))
                        for s in range(KS):
                            nc.tensor.matmul(
                                pstat[:, cp, 1:2],
                                xsq[:, s, cp * P:(cp + 1) * P], ones_col[:],
                                start=(s == 0), stop=(s == KS - 1))
                    stat2 = scr.tile([P, 2, 33], bf16, tag="stat2", bufs=2)
                    vcol = scr.tile([P, 2], f32, tag="vcol")
                    msq = scr.tile([P, 2], f32, tag="msq")
                    rcol = scr.tile([P, 2], f32, tag="rcol", bufs=2)
                    nc.scalar.activation(
                        stat2[:, :, 0], pstat[:, :, 0], AF.Copy, scale=-1.0 / C)
                    nc.vector.tensor_scalar(
                        vcol[:], pstat[:, :, 1], 1.0 / C, EPS_LN,
                        op0=ALU.mult, op1=ALU.add)
                    nc.scalar.activation(msq[:], stat2[:, :, 0], AF.Square)
                    nc.vector.tensor_sub(vcol[:], vcol[:], msq[:])
                    nc.scalar.activation(rcol[:], vcol[:], AF.Sqrt)
                    nc.vector.reciprocal(rcol[:], rcol[:])
                    nc.vector.tensor_copy(stat2[:, :, 32], rcol[:])
                    return stat2, rcol

                def stats_tr(t, stat2, cp):
                    sl = slice((t + cp) * P, (t + cp + 1) * P)
                    ptr = ps.tile([33, P], bf16, tag="pb")
                    nc.tensor.transpose(ptr[:], stat2[:, cp, :], identb[:])
                    nc.scalar.copy(mrow[0:1, sl], ptr[0:1, :])
                    nc.scalar.copy(rrow[0:1, sl], ptr[32:33, :])

                pend = None
                cur = stats_mm2(0)
                stats_tr(0, cur[0], 0)
                stats_tr(0, cur[0], 1)
                nxt = None
                for t in range(NT):
                    if 1 <= t <= len(interleave):
                        interleave[t - 1]()
                    sl = slice(t * P, (t + 1) * P)
                    rcol = cur[1][:, t % 2:t % 2 + 1]
                    if t % 2 == 0 and t + 2 < NT:
                        nxt = stats_mm2(t + 2)
                    # qkT x-part into PSUM (two banks)
                    pa1 = ps.tile([P, 512], f32, tag="pb")
                    pa2 = ps.tile([P, 256], f32, tag="pb")
                    for s in range(KS):
                        nc.tensor.matmul(
                            pa1[:], x_sb[:, s, sl], wqk_sb[:, s, 0:512],
                            start=(s == 0), stop=False)
                    for s in range(KS):
                        nc.tensor.matmul(
                            pa2[:], x_sb[:, s, sl], wqk_sb[:, s, 512:768],
                            start=(s == 0), stop=False)
                    if t % 2 == 0 and t + 2 < NT:
                        stats_tr(t + 2, nxt[0], 0)
                        stats_tr(t + 2, nxt[0], 1)
                    # rank-1 mean completion (rows for chunk t are ready)
                    nc.tensor.matmul(
                        pa1[:], mrow[0:1, sl], uqk_sb[0:1, 0:512],
                        start=False, stop=True)
                    nc.tensor.matmul(
                        pa2[:], mrow[0:1, sl], uqk_sb[0:1, 512:768],
                        start=False, stop=True)
                    # deferred S/norm accumulation from previous chunk
                    if pend is not None:
                        _emit_s(acc1, acc2, *pend)
                    # evictions: qk = rstd*pa (DVE + ACT), qksq = qk² (DVE)
                    qk = qkpool.tile([P, 2 * C], bf16, tag="qk")
                    qksq = qkpool.tile([P, 2 * C], bf16, tag="qksq")
                    nc.vector.tensor_scalar_mul(qk[:, 0:512], pa1[:], rcol)
                    nc.scalar.activation(
                        qk[:, 512:768], pa2[:], AF.Copy, scale=rcol)
                    nc.vector.tensor_mul(qksq[:], qk[:], qk[:])
                    pend = (qk, qksq, t)
                    if t % 2 == 1:
                        cur = nxt
                _emit_s(acc1, acc2, *pend)

            def _emit_s(acc1, acc2, qk, qksq, t):
                st, sp = (t == 0), (t == NT - 1)
                for h in range(NH):
                    o = h * 2 * CH
                    nc.tensor.matmul(
                        acc1[:, h * CH:(h + 1) * CH],
                        qk[:, o:o + CH], qk[:, o + CH:o + 2 * CH],
                        start=st, stop=sp)
                for h in range(NH):
                    o = h * 2 * CH
                    nc.tensor.matmul(
                        acc1[:, 384 + h:385 + h],
                        qksq[:, o:o + CH], ones_col[:],
                        start=st, stop=sp)
                ksq = qksq.rearrange("p (h two c) -> p h two c", two=2, c=CH)
                nc.tensor.matmul(
                    acc2[:], ones_col[:], ksq[:, :, 1, :], start=st, stop=sp)

            def attn_stages(img, acc1, acc2):
                """Softmax + G build as a list of emission closures."""
                st = {}

                def s0():  # norms + scaled S + softmax -> sSb (bf16)
                    rq = apool.tile([CH, NH], f32, tag="rq", bufs=1)
                    nc.scalar.activation(rq[:], acc1[:, 384:392], AF.Sqrt)
                    nc.vector.tensor_scalar_max(rq[:], rq[:], EPS_NORM)
                    nc.vector.reciprocal(rq[:], rq[:])
                    nc.vector.tensor_mul(rq[:], rq[:], schb[:])
                    rk = apool.tile([1, C], f32, tag="rk", bufs=1)
                    nc.scalar.activation(rk[:], acc2[:], AF.Sqrt)
                    nc.vector.tensor_scalar_max(rk[:], rk[:], EPS_NORM)
                    nc.vector.reciprocal(rk[:], rk[:])
                    rkb = apool.tile([1, C], bf16, tag="rkb", bufs=1)
                    nc.vector.tensor_copy(rkb[:], rk[:])
                    rkb_ps = ps.tile([CH, C], f32, tag="pb")
                    nc.tensor.matmul(
                        rkb_ps[:], ones_row[0:1, 0:CH], rkb[0:1, :],
                        start=True, stop=True)
                    sS = apool.tile([CH, NH, CH], f32, tag="sS", bufs=1)
                    s_v = acc1[:, 0:384].rearrange("p (h e) -> p h e", e=CH)
                    nc.vector.tensor_mul(
                        sS[:], s_v, rq[:, :, None].to_broadcast((CH, NH, CH)))
                    rkb_v = rkb_ps.rearrange("p (h e) -> p h e", e=CH)
                    nc.vector.tensor_mul(sS[:], sS[:], rkb_v)
                    mx = apool.tile([CH, NH], f32, tag="mx", bufs=1)
                    nc.vector.reduce_max(mx[:], sS[:], axis=AX.X)
                    nc.vector.tensor_sub(
                        sS[:], sS[:], mx[:, :, None].to_broadcast((CH, NH, CH)))
                    nc.scalar.activation(sS[:], sS[:], AF.Exp)
                    esum = apool.tile([CH, NH], f32, tag="esum", bufs=1)
                    nc.vector.reduce_sum(esum[:], sS[:], axis=AX.X)
                    nc.vector.reciprocal(esum[:], esum[:])
                    sSb = apool.tile([CH, NH, CH], bf16, tag="sSb", bufs=1)
                    nc.vector.tensor_mul(
                        sSb[:], sS[:],
                        esum[:, :, None].to_broadcast((CH, NH, CH)))
                    st["sSb"] = sSb

                def s1():  # transpose attn per head
                    pt8 = ps.tile([CH, NH, CH], bf16, tag="pb")
                    for h in range(NH):
                        nc.tensor.transpose(
                            pt8[:, h, :], st["sSb"][:, h, :], identb[0:CH, 0:CH])
                    atT = apool.tile([CH, NH, CH], bf16, tag="atT", bufs=1)
                    nc.vector.tensor_copy(atT[:], pt8[:])
                    st["atT"] = atT

                def s2():  # awv_h = attn_h @ Wv_h
                    awv = apool.tile([CH, NH, C], bf16, tag="awv", bufs=1)
                    for h in range(NH):
                        paw = ps.tile([CH, C], f32, tag="pb")
                        nc.tensor.matmul(
                            paw[:], st["atT"][:, h, :], wv_sb[:, h, :],
                            start=True, stop=True)
                        if h % 2 == 0:
                            nc.vector.tensor_copy(awv[:, h, :], paw[:])
                        else:
                            nc.scalar.copy(awv[:, h, :], paw[:])
                    st["awv"] = awv

                def s3():  # G^T
                    gt_sb = apool.tile([P, KS, C], bf16, tag="gt")
                    for j in range(KS):
                        pgt = ps.tile([P, C], f32, tag="pb")
                        for h in range(NH):
                            nc.tensor.matmul(
                                pgt[:], st["awv"][:, h, j * P:(j + 1) * P],
                                wpj_sb[:, h, :], start=(h == 0), stop=(h == NH - 1))
                        if j % 2 == 0:
                            nc.vector.tensor_copy(gt_sb[:, j, :], pgt[:])
                        else:
                            nc.scalar.copy(gt_sb[:, j, :], pgt[:])
                    st["gt"] = gt_sb

                def s4():  # uG row
                    pug = ps.tile([1, C], f32, tag="pb")
                    for s in range(KS):
                        nc.tensor.matmul(
                            pug[:], ones_col[:], st["gt"][:, s, :],
                            start=(s == 0), stop=(s == KS - 1))
                    ug = apool.tile([1, C], bf16, tag="ug")
                    nc.vector.tensor_copy(ug[:], pug[:])
                    st["ug"] = ug

                return [s0, s1, s2, s3, s4], st

            # per-image y / yn tiles (yn in fp8 for the DR ffn)
            f8sc = 64.0  # host scales w1/w2 by 64 (fp8 e4m3 denormal floor)

            def phase_b1(img, st, interleave=()):
                """G-branch apply + residual + LN2; produces y (bf16) and
                yn (fp8) for the whole image."""
                mrow, rrow = rowpairs[img]
                # y reuses the x image slots (x residency ends with phase A;
                # B1 re-reads x chunk-wise from DRAM)
                y = xpool.tile([P, KS, N], bf16, tag="x", name=f"y{img}")
                yn = bw.tile([P, KS, N], f8, tag="yn", bufs=1)
                gt, ug = st["gt"], st["ug"]
                pends, pend2 = [], []
                for f in range(NFG):
                    if f < len(interleave):
                        interleave[f]()
                    sl = slice(f * FG, (f + 1) * FG)
                    xb = scr.tile([P, KS, FG], bf16, tag="xb", bufs=2)
                    nc.sync.dma_start(xb[:], xs_r[img][:, :, sl])
                    bc1 = ps.tile([P, FG], f32, tag="pb")
                    nc.tensor.matmul(
                        bc1[:], ones_row[0:1, :], rrow[0:1, sl],
                        start=True, stop=True)
                    rb = scr.tile([P, FG], bf16, tag="rb", bufs=2)
                    nc.scalar.copy(rb[:], bc1[:])
                    pgs = []
                    for j in range(KS):
                        pg = ps.tile([P, FG], f32, tag="pb", name=f"pg{j}")
                        for s in range(KS):
                            nc.tensor.matmul(
                                pg[:], gt[:, s, j * P:(j + 1) * P],
                                xb[:, s, :], start=(s == 0), stop=False)
                        nc.tensor.matmul(
                            pg[:], ug[0:1, j * P:(j + 1) * P], mrow[0:1, sl],
                            start=False, stop=True)
                        pgs.append(pg)
                    ab = bw.tile([P, KS, FG], bf16, tag="ab", bufs=1)
                    for j in range(KS):
                        nc.vector.tensor_mul(ab[:, j, :], pgs[j][:], rb[:])
                    nc.vector.tensor_add(y[:, :, sl], xb[:], ab[:])
                    ysq = bw.tile([P, KS, FG], bf16, tag="ysq", bufs=2)
                    nc.gpsimd.tensor_mul(ysq[:], y[:, :, sl], y[:, :, sl])
                    # 2-deep pipeline: stats one chunk behind, apply two
                    if f >= 1:
                        pend2.append(_ln2_stats(img, y, f - 1, pends[f - 1]))
                    if f >= 2:
                        _ln2_apply(img, y, yn, f - 2, pend2[f - 2])
                    pends.append(ysq)
                pend2.append(_ln2_stats(img, y, NFG - 1, pends[NFG - 1]))
                _ln2_apply(img, y, yn, NFG - 2, pend2[NFG - 2])
                _ln2_apply(img, y, yn, NFG - 1, pend2[NFG - 1])
                return y, yn

            def _ln2_stats(img, y, f, ysq):
                sl = slice(f * FG, (f + 1) * FG)
                p2a = ps.tile([1, FG], f32, tag="pb")
                p2b = ps.tile([1, FG], f32, tag="pb")
                for s in range(KS):
                    nc.tensor.matmul(
                        p2a[:], ones_col[:], y[:, s, sl],
                        start=(s == 0), stop=(s == KS - 1))
                for s in range(KS):
                    nc.tensor.matmul(
                        p2b[:], ones_col[:], ysq[:, s, :],
                        start=(s == 0), stop=(s == KS - 1))
                m2b = scr.tile([1, FG], bf16, tag="m2b", bufs=2)
                nc.scalar.activation(m2b[:], p2a[:], AF.Copy, scale=-1.0 / C)
                vrow = scr.tile([1, FG], f32, tag="vrow", bufs=2)
                nc.vector.tensor_scalar(
                    vrow[:], p2b[:], 1.0 / C, EPS_LN, op0=ALU.mult, op1=ALU.add)
                msq = scr.tile([1, FG], f32, tag="msqr", bufs=2)
                nc.scalar.activation(msq[:], m2b[:], AF.Square)
                nc.vector.tensor_sub(vrow[:], vrow[:], msq[:])
                srow = scr.tile([1, FG], f32, tag="srow", bufs=2)
                nc.scalar.activation(srow[:], vrow[:], AF.Sqrt)
                r2f = scr.tile([1, FG], f32, tag="r2f", bufs=2)
                nc.vector.reciprocal(r2f[:], srow[:])
                r2b = scr.tile([1, FG], bf16, tag="r2b", bufs=2)
                nc.scalar.copy(r2b[:], r2f[:])
                return m2b, r2b

            def _ln2_apply(img, y, yn, f, rows):
                sl = slice(f * FG, (f + 1) * FG)
                m2b, r2b = rows
                bcm = ps.tile([P, FG], f32, tag="pb")
                nc.tensor.matmul(
                    bcm[:], ones_row[0:1, :], m2b[0:1, :], start=True, stop=True)
                bcr = ps.tile([P, FG], f32, tag="pb")
                nc.tensor.matmul(
                    bcr[:], ones_row[0:1, :], r2b[0:1, :], start=True, stop=True)
                mbc = scr.tile([P, FG], bf16, tag="mbc", bufs=2)
                nc.scalar.copy(mbc[:], bcm[:])
                rbc = scr.tile([P, FG], bf16, tag="rbc", bufs=2)
                nc.scalar.copy(rbc[:], bcr[:])
                t3 = bw.tile([P, KS, FG], bf16, tag="t3", bufs=1)
                nc.vector.tensor_add(
                    t3[:], y[:, :, sl], mbc[:, None, :].to_broadcast((P, KS, FG)))
                nc.vector.tensor_mul(
                    t3[:], t3[:], rbc[:, None, :].to_broadcast((P, KS, FG)))
                nc.scalar.copy(yn[:, :, sl], t3[:])

            def phase_b2(img, y, yn):
                """FFN in fp8 DoubleRow + residual + store."""
                for f in range(NFG):
                    sl = slice(f * FG, (f + 1) * FG)
                    h_sb = bw.tile([P, KH, FG], f8, tag="h")
                    for mh in range(KH):
                        ph = ps.tile([P, FG], f32, tag="pb")
                        nc.tensor.matmul(
                            ph[:], w1_sb[:, 0:2, mh * P:(mh + 1) * P],
                            yn[:, 0:2, sl], start=True, stop=False,
                            perf_mode=mybir.MatmulPerfMode.DoubleRow)
                        nc.tensor.matmul(
                            ph[:], w1_sb[:, 2, mh * P:(mh + 1) * P],
                            yn[:, 2, sl], start=False, stop=True)
                        nc.scalar.activation(
                            h_sb[:, mh, :], ph[:], AF.Gelu, scale=1.0 / f8sc)
                    o_sb = bw.tile([P, KS, FG], bf16, tag="o", bufs=1)
                    for mo in range(KS):
                        po = ps.tile([P, FG], f32, tag="pb")
                        for sp in range(KH // 2):
                            nc.tensor.matmul(
                                po[:], w2_sb[:, 2 * sp:2 * sp + 2,
                                             mo * P:(mo + 1) * P],
                                h_sb[:, 2 * sp:2 * sp + 2, :],
                                start=(sp == 0), stop=(sp == KH // 2 - 1),
                                perf_mode=mybir.MatmulPerfMode.DoubleRow)
                        nc.vector.scalar_tensor_tensor(
                            o_sb[:, mo, :], po[:], 1.0 / f8sc, y[:, mo, sl],
                            op0=ALU.mult, op1=ALU.add)
                    nc.sync.dma_start(out_r[img][:, :, sl], o_sb[:])

            # ----------------- schedule -----------------
            acc0 = alloc_acc()
            phase_a(0, acc0)
            stages0, st0 = attn_stages(0, *acc0)
            accB = alloc_acc()
            phase_a(1, accB, interleave=stages0)
            stages1, st1 = attn_stages(1, *accB)
            y0, yn0 = phase_b1(0, st0, interleave=stages1)
            phase_b2(0, y0, yn0)
            y1, yn1 = phase_b1(1, st1)
            phase_b2(1, y1, yn1)

    return _split_waits(nc)


def _prep_weights(inputs):
    import ml_dtypes
    bf = ml_dtypes.bfloat16
    f8 = ml_dtypes.float8_e4m3
    w_qkv = np.asarray(inputs["w_qkv"], np.float32)
    g1 = np.asarray(inputs["g1"], np.float32)
    g2 = np.asarray(inputs["g2"], np.float32)
    for name in ("beta1", "beta2", "b_qkv", "b_proj", "b_ffn1", "b_ffn2"):
        assert not np.any(np.asarray(inputs[name])), f"{name} nonzero unsupported"
    wg = w_qkv * g1[None, :]  # fold LN gamma into qkv weights
    wg3 = wg.reshape(NH, 3 * CH, C)
    wq = wg3[:, 0:CH, :]
    wk = wg3[:, CH:2 * CH, :]
    wv_ = wg3[:, 2 * CH:3 * CH, :]
    # qk columns interleaved per head: j = h*96 + (0..47 q | 48..95 k)
    wqk = np.concatenate([wq, wk], axis=1).reshape(2 * C, C)
    wqk_t = np.ascontiguousarray(wqk.T)  # [384, 768]
    u_qk = wqk.sum(axis=1)[None, :]  # [1, 768]
    wv_t = np.ascontiguousarray(wv_.transpose(1, 0, 2))  # [48, NH, 384]
    wpj48 = np.ascontiguousarray(
        np.asarray(inputs["w_proj"], np.float32).T.reshape(NH, CH, C)
        .transpose(1, 0, 2))
    w1g = np.asarray(inputs["w_ffn1"], np.float32) * g2[None, :]
    w1_t = np.ascontiguousarray(w1g.T)  # [384, 1536]
    w2_t = np.ascontiguousarray(np.asarray(inputs["w_ffn2"], np.float32).T)
    ls = np.asarray(inputs["logit_scale"], np.float32).reshape(NH)
    scale_row = np.exp(np.minimum(ls, LOGIT_MAX))[None, :]
    # ffn weights scaled by 64 into fp8 e4m3 (compensated at eviction) to
    # stay clear of the e4m3 denormal floor (2^-6)
    return dict(
        wqk_t=wqk_t.astype(bf), u_qk=np.ascontiguousarray(u_qk).astype(bf),
        wv=wv_t.astype(bf), wpj48=wpj48.astype(bf),
        w1_t=(w1_t * 64.0).astype(f8), w2_t=(w2_t * 64.0).astype(f8),
        scale_row=np.ascontiguousarray(scale_row).astype(np.float32))


def _make_in_maps(inputs):
    import ml_dtypes
    x = np.asarray(inputs["x"], np.float32).reshape(B, C, N).astype(
        ml_dtypes.bfloat16)
    wmap = _prep_weights(inputs)
    in_maps = []
    for c in range(NCORES):
        m = dict(wmap)
        m["xs"] = np.ascontiguousarray(x[c * BPC:(c + 1) * BPC])
        in_maps.append(m)
    return in_maps


def kernel(**inputs):
    from concourse.bass_utils import run_bass_kernel_spmd

    if "nc" not in _CACHE:
        _CACHE["nc"] = _build_nc()
    nc = _CACHE["nc"]
    in_maps = _make_in_maps(inputs)
    res = run_bass_kernel_spmd(nc, in_maps, list(range(NCORES)))
    out = np.concatenate(
        [np.asarray(r["out"], np.float32) for r in res.results], axis=0)
    return out.reshape(B, C, 64, 64)
